# revision 1
# baseline (speedup 1.0000x reference)
"""Trainium2 Bass kernel for a dense pre-LN transformer block (B=2, T=2048, C=1024, H=16).

Sharding: zero-collective sequence parallelism over 8 cores. Core c handles
batch b=c//4 and query tiles {r, 7-r, 8+r, 15-r} (r=c%4, 128 rows each) of
that batch: it computes LN1 on the full k/v of its batch, all 16 attention
heads for its 512 query rows, and the attention projection + full MLP for
those rows. The complementary tile assignment balances causal work, and the
program is identical on every core: slot i runs a fixed key-block loop of
length 4*(i+1), with per-core causal masks (triangular/zero blocks) supplied
as input data.

Numerics: LN and softmax accumulate in fp32; q/k transposed activations are
fp32r (PE matmul at full rate, ~1e-4 rounding); attention probabilities and
v are bf16 with fp32 PSUM accumulation; MLP weights/activations bf16 with
fp32 accumulation and fp32 layernorm/residuals.
"""

import sys

sys.path.insert(0, "/opt/trn_rl_repo")

import numpy as np
import ml_dtypes

import concourse.bass as bass
import concourse.bacc as bacc
import concourse.mybir as mybir
import concourse.tile as tile
from concourse.bass_utils import run_bass_kernel_spmd

F32 = mybir.dt.float32
F32R = mybir.dt.float32r
BF16 = mybir.dt.bfloat16
AF = mybir.ActivationFunctionType
ALU = mybir.AluOpType

B, T, C, H, D = 2, 2048, 1024, 16, 64
NT = T // 128          # 16 key tiles
NC = C // 128          # 8 channel tiles
NF = 4 * C // 128      # 32 fc tiles
NSLOT = 4              # query tiles per core
N_CORES = 8
EPS = 1e-5
SCALE = 1.0 / 8.0      # 1/sqrt(D)

_CACHE = {}


def build():
    nc = bacc.Bacc("TRN2", target_bir_lowering=False, debug=False,
                   num_devices=N_CORES)

    q_d = nc.dram_tensor("q_s", [NSLOT, 128, C], F32, kind="ExternalInput")
    k_d = nc.dram_tensor("k_f", [NT, 128, C], BF16, kind="ExternalInput")
    v_d = nc.dram_tensor("v_f", [NT, 128, C], BF16, kind="ExternalInput")
    mask_d = nc.dram_tensor("mask", [128, NSLOT, 4, 128], BF16, kind="ExternalInput")
    cpw_d = nc.dram_tensor("cpw_t", [C, C], BF16, kind="ExternalInput")
    fcw_d = nc.dram_tensor("fcw_t", [C, 4 * C], BF16, kind="ExternalInput")
    pjw_d = nc.dram_tensor("pjw_t", [4 * C, C], BF16, kind="ExternalInput")
    vecs_d = nc.dram_tensor("vecs", [C, 4], F32, kind="ExternalInput")
    w2b2_d = nc.dram_tensor("w2b2", [2, C], BF16, kind="ExternalInput")
    fcb_d = nc.dram_tensor("fcb", [4 * C], F32, kind="ExternalInput")
    w2f_d = nc.dram_tensor("w2b2f", [2, C], F32, kind="ExternalInput")
    out_d = nc.dram_tensor("out", [NSLOT, 128, C], F32, kind="ExternalOutput")

    with tile.TileContext(nc) as tc:
      with tc.tile_pool(name="pg", bufs=1) as pg:
        # ---- small constants / vectors (live whole kernel) ----
        vecs = pg.tile([128, NC, 4], F32)     # cols: ln1_w, ln1_b, apb, pjb
        nc.sync.dma_start(vecs[:], vecs_d.ap().rearrange("(ct p) v -> p ct v", p=128))
        w2b2 = pg.tile([1, 2, NC, 128], BF16)
        nc.sync.dma_start(w2b2[:], w2b2_d.ap().rearrange("k (ct p) -> k ct p", p=128)
                          .unsqueeze(0))
        fcb = pg.tile([128, NF], F32)
        nc.sync.dma_start(fcb[:], fcb_d.ap().rearrange("(ft p) -> p ft", p=128))
        w1_bf = pg.tile([1, C], BF16)
        nc.gpsimd.dma_start(w1_bf[:], vecs_d.ap()[:, 0:1].rearrange("c v -> v c"))

        ones_sb = pg.tile([128, 128], F32)
        nc.gpsimd.memset(ones_sb[:], 1.0)
        ident = pg.tile([128, 128], F32)
        nc.gpsimd.affine_select(ident[:], ones_sb[:], [[1, 128]], ALU.is_equal,
                                0.0, channel_multiplier=-1)
        ones_bf = pg.tile([128, 1], BF16)
        nc.gpsimd.memset(ones_bf[:], 1.0)
        ones128_bf = pg.tile([128, 128], BF16)
        nc.gpsimd.memset(ones128_bf[:], 1.0)
        ident_bf = pg.tile([128, 128], BF16)
        nc.gpsimd.affine_select(ident_bf[:], ones128_bf[:], [[1, 128]], ALU.is_equal,
                                0.0, channel_multiplier=-1)

        ln1w = lambda ct: vecs[:, ct, 0:1]
        ln1b = lambda ct: vecs[:, ct, 1:2]
        apb = lambda ct: vecs[:, ct, 2:3]
        pjb = lambda ct: vecs[:, ct, 3:4]

        # ---- cross-phase tensors ----
        qT = pg.tile([128, NC, 512], F32R)    # LN1(q)^T with w,b applied
        qT_bf = pg.tile([128, NC, 512], BF16)  # bf16 shadow for QK rhs
        xT = pg.tile([128, NC, 512], F32)     # attn residual output (C-major)

        py_cm = tc.tile_pool(name="py", bufs=1)
        py = py_cm.__enter__()
        yT_all = py.tile([128, NC, 512], F32)  # raw attention out (pre 1/s, w1, b1)
        s_bf = py.tile([1, H * 512], BF16)     # softmax denominator reciprocals
        s_all = py.tile([H, 512], F32)         # softmax denominators

        with tc.tile_pool(name="pa", bufs=1) as pa:
            kT = pa.tile([128, NC, T], BF16)       # LN1(k)^T with w,b
            v_ext = pa.tile([128, NT, H, 65], BF16)  # LN1(v) (no w,b) + ones col
            masks = pa.tile([128, NSLOT, 4, 128], BF16)
            nc.sync.dma_start(masks[:], mask_d.ap())

            # ---- LN stats pass: all 36 tiles (4 q, 16 k, 16 v), sqrt batched once ----
            NLN = NSLOT + 2 * NT
            aggr_all = pa.tile([128, NLN, 2], F32)
            rstd_all = pa.tile([128, NLN], F32)
            nmr_all = pa.tile([128, NLN], F32)

            # ========= Phase 1 (LN1) interleaved with Phase 2 (attention) =========
            with (
                tc.tile_pool(name="pln", bufs=3) as pl,
                tc.tile_pool(name="plz", bufs=1) as plz,
                tc.tile_pool(name="paw", bufs=2) as aw,
                tc.tile_pool(name="pap", bufs=2, space="PSUM") as aps,
            ):
                # pass A: stats only (x_in released immediately)
                def stats_one(src_d, tt, idx, dt_in):
                    x_in = pl.tile([128, C], dt_in, tag="ln_in" + ("b" if dt_in == BF16 else ""))
                    nc.sync.dma_start(x_in[:], src_d.ap()[tt])
                    stats = pl.tile([128, 2, 6], F32, tag="lns")
                    nc.vector.bn_stats(stats[:, 0, :], x_in[:, 0:512])
                    nc.vector.bn_stats(stats[:, 1, :], x_in[:, 512:1024])
                    nc.vector.bn_aggr(aggr_all[:, idx, :], stats[:])

                for i in range(NSLOT):
                    stats_one(q_d, i, i, F32)
                for tt in range(NT):
                    stats_one(k_d, tt, NSLOT + tt, BF16)
                for tt in range(NT):
                    stats_one(v_d, tt, NSLOT + NT + tt, BF16)

                veps = pa.tile([128, NLN], F32)
                nc.vector.tensor_scalar(veps[:], aggr_all[:, :, 1], EPS, None, ALU.add)
                nc.scalar.activation(rstd_all[:], veps[:], AF.Sqrt)
                nc.vector.reciprocal(rstd_all[:], rstd_all[:])
                nc.vector.tensor_tensor(nmr_all[:], aggr_all[:, :, 0], rstd_all[:],
                                        ALU.mult)
                nc.vector.tensor_scalar(nmr_all[:], nmr_all[:], -1.0, None, ALU.mult)

                # pass B: normalize + transpose
                def ln_transpose_group(src_d, tts, idx0, dstT, dst_off, also_bf=None,
                                       dt_in=F32):
                    zs = []
                    for gi, tt in enumerate(tts):
                        x_in = pl.tile([128, C], dt_in, tag="ln_in" + ("b" if dt_in == BF16 else ""))
                        nc.sync.dma_start(x_in[:], src_d.ap()[tt])
                        z = plz.tile([128, C], dt_in, tag=f"z{gi}" + ("b" if dt_in == BF16 else ""))
                        i = idx0 + gi
                        nc.scalar.activation(z[:], x_in[:], AF.Identity,
                                             bias=nmr_all[:, i:i + 1],
                                             scale=rstd_all[:, i:i + 1])
                        zs.append(z)
                    for ct in range(NC):
                        ps = aps.tile([128, 4, 128], F32, tag="tp")
                        if dt_in == BF16:
                            pv = ps[:].bitcast(BF16)[:, :, 0:128]
                        else:
                            pv = ps[:]
                        for gi in range(4):
                            nc.tensor.transpose(pv[:, gi, :],
                                                zs[gi][:, ct * 128:(ct + 1) * 128],
                                                ident_bf[:] if dt_in == BF16 else ident[:])
                        nc.vector.tensor_scalar(
                            dstT[:, ct, dst_off:dst_off + 512], pv[:],
                            ln1w(ct), ln1b(ct), ALU.mult, ALU.add)
                        if also_bf is not None:
                            nc.vector.tensor_copy(also_bf[:, ct, :],
                                                  dstT[:, ct, :].bitcast(F32))

                def ln_v_grp(tts):
                    for tt in tts:
                        x_in = pl.tile([128, C], BF16, tag="v_in")
                        nc.sync.dma_start(x_in[:], v_d.ap()[tt])
                        i = NSLOT + NT + tt
                        nc.gpsimd.memset(v_ext[:, tt, :, 64:65], 1.0)
                        nc.scalar.activation(v_ext[:, tt, :, 0:64],
                                             x_in[:].rearrange("p (h d) -> p h d", h=H),
                                             AF.Identity, bias=nmr_all[:, i:i + 1],
                                             scale=rstd_all[:, i:i + 1])

                def attn_pair(h, np_, c0, c1, nfrom):
                    ct, sel = h // 2, (h % 2) * 64
                    yp = aps.tile([65, 256], F32, tag="yp")
                    for ch in range(np_ // 4):
                        pbase = ch * 4
                        off = 0 if pbase < nfrom else 128
                        sc = aps.tile([128, 4, 256], F32, tag="sc")
                        att = aw.tile([128, 4, 256], BF16, tag="att")
                        for pc in range(4):
                            p = pbase + pc
                            nc.tensor.matmul(
                                sc[:, pc, :],
                                kT[sel:sel + 64, ct, p * 128:(p + 1) * 128],
                                qT_bf[sel:sel + 64, ct, c0:c1],
                                tile_position=(sel, 0),
                                skip_group_check=True)
                        nc.scalar.activation(att[:, :, off:256], sc[:, :, off:256],
                                             AF.Exp, scale=SCALE)
                        for i in range(NSLOT):
                            if c0 <= i * 128 < c1 and i * 4 == pbase:
                                acol = i * 128 - c0
                                nc.vector.tensor_tensor(
                                    att[:, :, acol:acol + 128],
                                    att[:, :, acol:acol + 128],
                                    masks[:, i, :, :],
                                    ALU.mult)
                        for pc in range(4):
                            p = pbase + pc
                            nc.tensor.matmul(
                                yp[:, off:256],
                                v_ext[:, p, h, :],
                                att[:, pc, off:256],
                                start=(p == 0), stop=(p == np_ - 1),
                                skip_group_check=True)
                    st = aw.tile([65, 256], F32, tag="st")
                    nc.vector.tensor_copy(st[:, :], yp[:, :])
                    nc.vector.tensor_copy(yT_all[sel:sel + 64, ct, c0:c1],
                                          st[0:64, :])
                    nc.sync.dma_start(s_all[h:h + 1, c0:c1], st[64:65, :])

                ln_transpose_group(q_d, range(NSLOT), 0, qT, 0, also_bf=qT_bf)
                for tg in range(2):
                    ln_transpose_group(k_d, range(tg * 4, tg * 4 + 4),
                                       NSLOT + tg * 4, kT, tg * 512, dt_in=BF16)
                    ln_v_grp(range(tg * 4, tg * 4 + 4))
                for h in range(H):
                    attn_pair(h, 8, 0, 256, 4)
                for tg in range(2, 4):
                    ln_transpose_group(k_d, range(tg * 4, tg * 4 + 4),
                                       NSLOT + tg * 4, kT, tg * 512, dt_in=BF16)
                    ln_v_grp(range(tg * 4, tg * 4 + 4))
                for h in range(H):
                    attn_pair(h, 16, 256, 512, 12)
                nc.vector.reciprocal(s_all[:], s_all[:])
                srec_b = py.tile([H, 512], BF16)
                nc.vector.tensor_copy(srec_b[:], s_all[:])
                for h in range(H):
                    nc.sync.dma_start(s_bf[0:1, h * 512:(h + 1) * 512],
                                      srec_b[h:h + 1, :])

        # ---- y scale + c_proj + residual -> xT ----
        with (
            tc.tile_pool(name="pc", bufs=1) as pcp,
            tc.tile_pool(name="pcw", bufs=3) as cw,
            tc.tile_pool(name="pcps", bufs=2, space="PSUM") as cps,
        ):
            cpwT = pcp.tile([128, NC, C], BF16)
            nc.sync.dma_start(cpwT[:], cpw_d.ap().rearrange("(ct p) o -> p ct o", p=128))
            ysc = pcp.tile([128, NC, 512], BF16)
            for ct in range(NC):
                rb = cps.tile([128, 512], F32, tag="rb")
                for half in range(2):
                    h = ct * 2 + half
                    for (c0, c1) in ((0, 256), (256, 512)):
                        nc.tensor.matmul(
                            rb[half * 64:half * 64 + 64, c0:c1],
                            w1_bf[0:1, h * 64:h * 64 + 64],
                            s_bf[0:1, h * 512 + c0:h * 512 + c1],
                            tile_position=(0, half * 64),
                            skip_group_check=True)
                t1 = cw.tile([128, 512], F32, tag="yt1")
                nc.vector.tensor_tensor(t1[:], yT_all[:, ct, :], rb[:], ALU.mult)
                nc.vector.tensor_scalar(ysc[:, ct, :], t1[:], 1.0, ln1b(ct),
                                        ALU.mult, ALU.add)
            for ot in range(NC):
                pj = cps.tile([128, 512], F32, tag="cp")
                for ct in range(NC):
                    nc.tensor.matmul(pj[:], cpwT[:, ct, ot * 128:(ot + 1) * 128],
                                     ysc[:, ct, :], start=(ct == 0),
                                     stop=(ct == NC - 1))
                t2 = cw.tile([128, 512], F32, tag="cpt")
                nc.scalar.activation(t2[:], pj[:], AF.Identity, bias=apb(ot))
                nc.vector.tensor_tensor(xT[:, ot, :], t2[:],
                                        qT[:, ot, :].bitcast(F32), ALU.add)

        py_cm.__exit__(None, None, None)

        # ================= Phase 3: LN2 + MLP =================
        with (
            tc.tile_pool(name="pm", bufs=1) as pm,
            tc.tile_pool(name="pmw", bufs=3) as mw,
            tc.tile_pool(name="pms", bufs=1, space="PSUM") as mps,
            tc.tile_pool(name="pma", bufs=2, space="PSUM") as mac,
        ):
            # LN2 stats via PE ones-reductions (partition-dim sums)
            s1 = mps.tile([1, 512], F32, tag="s1")
            s2 = mps.tile([1, 512], F32, tag="s2")
            for ct in range(NC):
                nc.tensor.matmul(s1[:], ones_sb[:, 0:1], xT[:, ct, :],
                                 start=(ct == 0), stop=(ct == NC - 1),
                                 skip_group_check=True)
            for ct in range(NC):
                sq = mw.tile([128, 512], BF16, tag="sq")
                nc.scalar.activation(sq[:], xT[:, ct, :], AF.Square)
                nc.tensor.matmul(s2[:], ones_bf[:], sq[:],
                                 start=(ct == 0), stop=(ct == NC - 1),
                                 skip_group_check=True)
            mu = pm.tile([1, 512], F32)
            nc.vector.tensor_scalar(mu[:], s1[:], 1.0 / C, None, ALU.mult)
            ex2 = pm.tile([1, 512], F32)
            nc.vector.tensor_scalar(ex2[:], s2[:], 1.0 / C, EPS, ALU.mult, ALU.add)
            var = pm.tile([1, 512], F32)
            nc.vector.tensor_tensor(var[:], mu[:], mu[:], ALU.mult)
            nc.vector.tensor_tensor(var[:], ex2[:], var[:], ALU.subtract)
            rstd2 = pm.tile([1, 512], F32)
            nc.scalar.activation(rstd2[:], var[:], AF.Sqrt)
            nc.vector.reciprocal(rstd2[:], rstd2[:])
            nmr2 = pm.tile([1, 512], F32)
            nc.vector.tensor_tensor(nmr2[:], mu[:], rstd2[:], ALU.mult)
            nc.vector.tensor_scalar(nmr2[:], nmr2[:], -1.0, None, ALU.mult)

            rstd2b = pm.tile([1, 512], BF16)
            nc.vector.tensor_copy(rstd2b[:], rstd2[:])
            nmr2b = pm.tile([1, 512], BF16)
            nc.vector.tensor_copy(nmr2b[:], nmr2[:])
            ones_bcol = pm.tile([1, 128], BF16)
            nc.gpsimd.memset(ones_bcol[:], 1.0)

            # broadcast rstd2 / -mu*rstd2 to all partitions once via PE
            zA = mps.tile([128, 512], F32, tag="zA")
            zB = mps.tile([128, 512], F32, tag="zB")
            nc.tensor.matmul(zA[:], ones_bcol[:], rstd2b[:], skip_group_check=True)
            nc.tensor.matmul(zB[:], ones_bcol[:], nmr2b[:], skip_group_check=True)

            # z2 = (x * rstd_bc + nmr_bc) * w2[c] + b2[c], bf16
            w2sb = pm.tile([128, NC, 2], F32)
            nc.sync.dma_start(w2sb[:, :, 0:1],
                              w2f_d.ap()[0:1, :].rearrange("k (ct p) -> p ct k", p=128))
            nc.sync.dma_start(w2sb[:, :, 1:2],
                              w2f_d.ap()[1:2, :].rearrange("k (ct p) -> p ct k", p=128))
            z2 = pm.tile([128, NC, 512], BF16)
            for ct in range(NC):
                t1 = mw.tile([128, 512], F32, tag="z2t")
                nc.vector.tensor_tensor(t1[:], xT[:, ct, :], zA[:], ALU.mult)
                nc.vector.tensor_tensor(t1[:], t1[:], zB[:], ALU.add)
                nc.vector.tensor_scalar(z2[:, ct, :], t1[:], w2sb[:, ct, 0:1],
                                        w2sb[:, ct, 1:2], ALU.mult, ALU.add)

            # fc + gelu -> mid (bf16)
            mid = pm.tile([128, NF, 512], BF16)
            for ft in range(NF):
                fw = mw.tile([128, NC, 128], BF16, tag="fw")
                nc.sync.dma_start(fw[:], fcw_d.ap()[:, ft * 128:(ft + 1) * 128]
                                  .rearrange("(ct p) f -> p ct f", p=128))
                fp = mac.tile([128, 512], F32, tag="acc")
                for ct in range(NC):
                    nc.tensor.matmul(fp[:], fw[:, ct, :], z2[:, ct, :],
                                     start=(ct == 0), stop=(ct == NC - 1))
                nc.scalar.activation(mid[:, ft, :], fp[:], AF.Gelu_apprx_tanh,
                                     bias=fcb[:, ft:ft + 1])

            # proj + pjb + residual -> outT
            outT = pm.tile([128, NC, 512], F32)
            for ot in range(NC):
                pw = mw.tile([128, NF, 128], BF16, tag="pw")
                nc.sync.dma_start(pw[:], pjw_d.ap()[:, ot * 128:(ot + 1) * 128]
                                  .rearrange("(ft p) f -> p ft f", p=128))
                pacc = mac.tile([128, 512], F32, tag="acc")
                for ft in range(NF):
                    nc.tensor.matmul(pacc[:], pw[:, ft, :], mid[:, ft, :],
                                     start=(ft == 0), stop=(ft == NF - 1))
                t3 = mw.tile([128, 512], F32, tag="ot3")
                nc.vector.tensor_scalar(t3[:], pacc[:], 1.0, pjb(ot),
                                        ALU.mult, ALU.add)
                nc.vector.tensor_tensor(outT[:, ot, :], t3[:], xT[:, ot, :], ALU.add)

            # transpose back to token-major and store
            for i in range(NSLOT):
                on = mw.tile([128, C], F32, tag="onat")
                for og in range(2):
                    po = mac.tile([128, 512], F32, tag="po")
                    for j in range(4):
                        ot = og * 4 + j
                        nc.tensor.transpose(po[:, j * 128:(j + 1) * 128],
                                            outT[:, ot, i * 128:(i + 1) * 128],
                                            ident[:])
                    nc.scalar.copy(on[:, og * 512:(og + 1) * 512], po[:])
                nc.sync.dma_start(out_d.ap()[i], on[:])

    nc.compile()
    return nc


def _host_prep(inputs):
    q = np.asarray(inputs["q"], np.float32)
    k = np.asarray(inputs["k"], np.float32)
    v = np.asarray(inputs["v"], np.float32)
    cpw_t = np.ascontiguousarray(np.asarray(inputs["attn_proj_w"], np.float32).T
                                 ).astype(ml_dtypes.bfloat16)
    fcw_t = np.ascontiguousarray(np.asarray(inputs["fc_w"], np.float32).T
                                 ).astype(ml_dtypes.bfloat16)
    pjw_t = np.ascontiguousarray(np.asarray(inputs["proj_w"], np.float32).T
                                 ).astype(ml_dtypes.bfloat16)
    vecs = np.ascontiguousarray(np.stack(
        [np.asarray(inputs["ln1_w"], np.float32),
         np.asarray(inputs["ln1_b"], np.float32),
         np.asarray(inputs["attn_proj_b"], np.float32),
         np.asarray(inputs["proj_b"], np.float32)], axis=1))
    w2b2f = np.ascontiguousarray(np.stack(
        [np.asarray(inputs["ln2_w"], np.float32),
         np.asarray(inputs["ln2_b"], np.float32)], axis=0))
    w2b2 = w2b2f.astype(ml_dtypes.bfloat16)
    fcb = np.ascontiguousarray(np.asarray(inputs["fc_b"], np.float32))

    tri = (np.arange(128)[:, None] <= np.arange(128)[None, :])  # keep tk<=tq

    in_maps, slot_map = [], []
    for c in range(N_CORES):
        b, r = c // 4, c % 4
        slots = [r, 7 - r, 8 + r, 15 - r]
        slot_map.append((b, slots))
        qs = q[b].reshape(NT, 128, C)[slots]
        mask = np.zeros((128, NSLOT, 4, 128), np.float32)
        for i, a in enumerate(slots):
            for p4 in range(4):
                p = 4 * i + p4
                if p < a:
                    mask[:, i, p4, :] = 1.0
                elif p == a:
                    mask[:, i, p4, :] = tri
        in_maps.append({
            "w2b2f": w2b2f,
            "q_s": np.ascontiguousarray(qs),
            "k_f": np.ascontiguousarray(k[b].reshape(NT, 128, C)).astype(ml_dtypes.bfloat16),
            "v_f": np.ascontiguousarray(v[b].reshape(NT, 128, C)).astype(ml_dtypes.bfloat16),
            "mask": mask.astype(ml_dtypes.bfloat16),
            "cpw_t": cpw_t, "fcw_t": fcw_t, "pjw_t": pjw_t,
            "vecs": vecs, "w2b2": w2b2, "fcb": fcb,
        })
    return in_maps, slot_map


def kernel(**inputs):
    if "nc" not in _CACHE:
        _CACHE["nc"] = build()
    nc = _CACHE["nc"]
    in_maps, slot_map = _host_prep(inputs)
    res = run_bass_kernel_spmd(nc, in_maps, core_ids=list(range(N_CORES)))
    out = np.empty((B, T, C), np.float32)
    for c in range(N_CORES):
        b, slots = slot_map[c]
        o = res.results[c]["out"]
        for i, a in enumerate(slots):
            out[b, a * 128:(a + 1) * 128, :] = o[i]
    return out



# revision 8
# speedup vs baseline: 1.2862x; 1.2862x over previous
"""Trainium2 Bass kernel for a dense pre-LN transformer block (B=2, T=2048, C=1024, H=16).

Sharding: zero-collective sequence parallelism over 8 cores. Core c handles
batch b=c//4 and query tiles {r, 7-r, 8+r, 15-r} (r=c%4, 128 rows each) of
that batch: it computes LN1 on the full k/v of its batch, all 16 attention
heads for its 512 query rows, and the attention projection + full MLP for
those rows. The complementary tile assignment balances causal work, and the
program is identical on every core.

Pipeline: per-group (4-tile) LN stats->normalize->transpose chains feed the
attention passes as soon as their key/value groups land, with PE warmup
bursts to trip the HAM clock gate to 8/8 early. ln1_w is folded into the
q-side operand and ln1_b dropped from the k-side (a per-query additive
constant cancels in softmax), so k/v normalization runs on the Vector engine
in bf16 4x mode. Softmax denominators are gathered per head-pair during the
second attention pass. MLP weights stream on the Sync DMA queue while small
gathers use the GpSimd (SWDGE) queue to avoid head-of-line blocking.
"""

import sys

sys.path.insert(0, "/opt/trn_rl_repo")

import numpy as np
import ml_dtypes

import concourse.bass as bass
import concourse.bacc as bacc
import concourse.mybir as mybir
import concourse.tile as tile
from concourse.bass_utils import run_bass_kernel_spmd

F32 = mybir.dt.float32
BF16 = mybir.dt.bfloat16
AF = mybir.ActivationFunctionType
ALU = mybir.AluOpType

B, T, C, H, D = 2, 2048, 1024, 16, 64
NT = T // 128          # 16 key tiles
NC = C // 128          # 8 channel tiles
NF = 4 * C // 128      # 32 fc tiles
NSLOT = 4              # query tiles per core
N_CORES = 8
EPS = 1e-5
SCALE = 1.0 / 8.0      # 1/sqrt(D)

_CACHE = {}


def build():
    nc = bacc.Bacc("TRN2", target_bir_lowering=False, debug=False,
                   num_devices=N_CORES)

    q_d = nc.dram_tensor("q_s", [NSLOT, 128, C], BF16, kind="ExternalInput")
    k_d = nc.dram_tensor("k_f", [NT, 128, C], BF16, kind="ExternalInput")
    v_d = nc.dram_tensor("v_f", [NT, 128, C], BF16, kind="ExternalInput")
    mask_d = nc.dram_tensor("mask", [128, NSLOT, 4, 128], BF16, kind="ExternalInput")
    cpw_d = nc.dram_tensor("cpw_t", [C, C], BF16, kind="ExternalInput")
    fcw_d = nc.dram_tensor("fcw_t", [C, 4 * C], BF16, kind="ExternalInput")
    pjw_d = nc.dram_tensor("pjw_t", [4 * C, C], BF16, kind="ExternalInput")
    vecs_d = nc.dram_tensor("vecs", [C, 4], F32, kind="ExternalInput")
    fcb_d = nc.dram_tensor("fcb", [4 * C], F32, kind="ExternalInput")
    w2f_d = nc.dram_tensor("w2b2f", [2, C], F32, kind="ExternalInput")
    out_d = nc.dram_tensor("out", [NSLOT, 128, C], F32, kind="ExternalOutput")

    with tile.TileContext(nc) as tc:
      with tc.tile_pool(name="pg", bufs=1) as pg:
        # ---- small constants / vectors (live whole kernel) ----
        vecs = pg.tile([128, NC, 4], F32)     # cols: ln1_w, ln1_b, apb, pjb
        nc.gpsimd.dma_start(vecs[:], vecs_d.ap().rearrange("(ct p) v -> p ct v", p=128))
        fcb = pg.tile([128, NF], F32)
        nc.gpsimd.dma_start(fcb[:], fcb_d.ap().rearrange("(ft p) -> p ft", p=128))
        w1_bf = pg.tile([1, C], BF16)
        nc.gpsimd.dma_start(w1_bf[:], vecs_d.ap()[:, 0:1].rearrange("c v -> v c"))
        w2sb = pg.tile([128, NC, 2], F32)
        nc.gpsimd.dma_start(w2sb[:, :, 0:1],
                            w2f_d.ap()[0:1, :].rearrange("k (ct p) -> p ct k", p=128))
        nc.gpsimd.dma_start(w2sb[:, :, 1:2],
                            w2f_d.ap()[1:2, :].rearrange("k (ct p) -> p ct k", p=128))

        ones_sb = pg.tile([128, 128], F32)
        nc.gpsimd.memset(ones_sb[:], 1.0)
        ident = pg.tile([128, 128], F32)
        nc.gpsimd.affine_select(ident[:], ones_sb[:], [[1, 128]], ALU.is_equal,
                                0.0, channel_multiplier=-1)
        ones_bf = pg.tile([128, 1], BF16)
        nc.gpsimd.memset(ones_bf[:], 1.0)
        ones128_bf = pg.tile([128, 128], BF16)
        nc.gpsimd.memset(ones128_bf[:], 1.0)
        ident_bf = pg.tile([128, 128], BF16)
        nc.gpsimd.affine_select(ident_bf[:], ones128_bf[:], [[1, 128]], ALU.is_equal,
                                0.0, channel_multiplier=-1)
        ones512_bf = pg.tile([128, 512], BF16)
        nc.gpsimd.memset(ones512_bf[:], 1.0)

        ln1w = lambda ct: vecs[:, ct, 0:1]
        ln1b = lambda ct: vecs[:, ct, 1:2]
        apb = lambda ct: vecs[:, ct, 2:3]
        pjb = lambda ct: vecs[:, ct, 3:4]

        # ---- cross-phase tensors ----
        qT = pg.tile([128, NC, 512], F32)     # LN1(q)^T with w,b (residual; reused as outT)
        qT2 = pg.tile([128, NC, 512], BF16)   # w * LN1(q)^T  (QK rhs; absorbs k-side w)
        xT = pg.tile([128, NC, 512], F32)     # attn residual output (C-major)
        cpwT = pg.tile([128, NC, C], BF16)

        py_cm = tc.tile_pool(name="py", bufs=1)
        py = py_cm.__enter__()
        yT_all = py.tile([128, NC, 512], F32)  # raw attention out (pre 1/s, w1, b1)
        s_all = py.tile([H, 512], F32)         # softmax denominators
        srec_b = py.tile([H, 512], BF16)
        s_bf = py.tile([1, H * 512], BF16)     # denominator reciprocals, head-major

        with tc.tile_pool(name="pa", bufs=1) as pa:
            kT = pa.tile([128, NC, T], BF16)       # LN1(k)^T, no w/b (folded to q side)
            v_ext = pa.tile([128, NT, H, 65], BF16)  # LN1(v) (no w,b) + ones col
            masks = pa.tile([128, NSLOT, 4, 128], BF16)
            nc.gpsimd.dma_start(masks[:], mask_d.ap())

            # warmup burst A: real-rate bf16 matmuls to trip HAM to 8/8 early
            with tc.tile_pool(name="wps", bufs=1, space="PSUM") as wps:
                wu = wps.tile([128, 512], F32, tag="wu")
                for _ in range(16):
                    nc.tensor.matmul(wu[:], ones128_bf[:], ones512_bf[:],
                                     skip_group_check=True)

            with (
                tc.tile_pool(name="pln", bufs=9) as pl,
                tc.tile_pool(name="plz", bufs=2) as pz,
                tc.tile_pool(name="pls", bufs=2) as pstat,
                tc.tile_pool(name="paw", bufs=2) as aw,
                tc.tile_pool(name="pap", bufs=2, space="PSUM") as aps,
            ):
                def ln_group(src_d, tts, kind):
                    # load 4 tiles, stats, batched rsqrt, normalize (DVE bf16 4x)
                    xs = []
                    for tt in tts:
                        x_in = pl.tile([128, C], BF16, tag="ln_kv")
                        nc.sync.dma_start(x_in[:], src_d.ap()[tt])
                        xs.append(x_in)
                    aggr = pstat.tile([128, 4, 2], F32, tag="aggr")
                    for gi in range(4):
                        st2 = pstat.tile([128, 2, 6], F32, tag="st2")
                        nc.vector.bn_stats(st2[:, 0, :], xs[gi][:, 0:512])
                        nc.vector.bn_stats(st2[:, 1, :], xs[gi][:, 512:1024])
                        nc.vector.bn_aggr(aggr[:, gi, :], st2[:])
                    veps = pstat.tile([128, 4], F32, tag="veps")
                    rstd = pstat.tile([128, 4], F32, tag="rstd")
                    nmr = pstat.tile([128, 4], F32, tag="nmr")
                    nc.vector.tensor_scalar(veps[:], aggr[:, :, 1], EPS, None, ALU.add)
                    nc.scalar.activation(rstd[:], veps[:], AF.Sqrt)
                    nc.vector.reciprocal(rstd[:], rstd[:])
                    nc.vector.tensor_tensor(nmr[:], aggr[:, :, 0], rstd[:], ALU.mult)
                    nc.vector.tensor_scalar(nmr[:], nmr[:], -1.0, None, ALU.mult)

                    if kind == "v":
                        for gi, tt in enumerate(tts):
                            nc.gpsimd.memset(v_ext[:, tt, :, 64:65], 1.0)
                            nc.vector.tensor_scalar(
                                v_ext[:, tt, :, 0:64],
                                xs[gi][:].rearrange("p (h d) -> p h d", h=H),
                                rstd[:, gi:gi + 1], nmr[:, gi:gi + 1],
                                ALU.mult, ALU.add)
                        return

                    zs = []
                    for gi in range(4):
                        z = pz.tile([128, C], BF16, tag=f"z{gi}")
                        nc.vector.tensor_scalar(z[:], xs[gi][:], rstd[:, gi:gi + 1],
                                                nmr[:, gi:gi + 1], ALU.mult, ALU.add)
                        zs.append(z)
                    for ct in range(NC):
                        ps = aps.tile([128, 4, 128], F32, tag="tp")
                        pv = ps[:].bitcast(BF16)[:, :, 0:128]
                        for gi in range(4):
                            nc.tensor.transpose(pv[:, gi, :],
                                                zs[gi][:, ct * 128:(ct + 1) * 128],
                                                ident_bf[:])
                        if kind == "q":
                            nc.vector.tensor_scalar(qT[:, ct, :], pv[:],
                                                    ln1w(ct), ln1b(ct),
                                                    ALU.mult, ALU.add)
                            nc.vector.tensor_scalar(qT2[:, ct, :], qT[:, ct, :],
                                                    ln1w(ct), None, ALU.mult)
                        else:  # k
                            nc.vector.tensor_copy(
                                kT[:, ct, tts[0] * 128:(tts[0] + 4) * 128], pv[:])

                def attn_pair(h, np_, c0, c1, nfrom):
                    ct, sel = h // 2, (h % 2) * 64
                    yp = aps.tile([65, 256], F32, tag="yp")
                    for ch in range(np_ // 4):
                        pbase = ch * 4
                        off = 0 if pbase < nfrom else 128
                        sc = aps.tile([128, 4, 256], F32, tag="sc")
                        att = aw.tile([128, 4, 256], BF16, tag="att")
                        for pc in range(4):
                            p = pbase + pc
                            nc.tensor.matmul(
                                sc[:, pc, :],
                                kT[sel:sel + 64, ct, p * 128:(p + 1) * 128],
                                qT2[sel:sel + 64, ct, c0:c1],
                                tile_position=(sel, 0),
                                skip_group_check=True)
                        nc.scalar.activation(att[:, :, off:256], sc[:, :, off:256],
                                             AF.Exp, scale=SCALE)
                        for i in range(NSLOT):
                            if c0 <= i * 128 < c1 and i * 4 == pbase:
                                acol = i * 128 - c0
                                nc.vector.tensor_tensor(
                                    att[:, :, acol:acol + 128],
                                    att[:, :, acol:acol + 128],
                                    masks[:, i, :, :],
                                    ALU.mult)
                        for pc in range(4):
                            p = pbase + pc
                            nc.tensor.matmul(
                                yp[:, off:256],
                                v_ext[:, p, h, :],
                                att[:, pc, off:256],
                                start=(p == 0), stop=(p == np_ - 1),
                                skip_group_check=True)
                    st = aw.tile([65, 256], F32, tag="sst")
                    nc.vector.tensor_copy(st[:, :], yp[:, :])
                    nc.vector.tensor_copy(yT_all[sel:sel + 64, ct, c0:c1],
                                          st[0:64, :])
                    nc.gpsimd.dma_start(s_all[h:h + 1, c0:c1], st[64:65, :])

                # ---- pipeline ----
                ln_group(q_d, range(NSLOT), "q")
                ln_group(k_d, range(0, 4), "k")
                ln_group(k_d, range(4, 8), "k")
                ln_group(v_d, range(0, 4), "v")
                ln_group(v_d, range(4, 8), "v")

                # warmup burst B: re-trip HAM right before the QK/AV stream
                wub = aps.tile([128, 4, 256], F32, tag="sc")
                for _ in range(20):
                    nc.tensor.matmul(wub[:, 0:2, :], ones128_bf[:], ones512_bf[:],
                                     skip_group_check=True)

                for h in range(H):
                    attn_pair(h, 8, 0, 256, 4)

                # pass-1 denominator half: reciprocal overlaps pass 2
                nc.vector.reciprocal(s_all[:, 0:256], s_all[:, 0:256])
                nc.vector.tensor_copy(srec_b[:, 0:256], s_all[:, 0:256])
                for h in range(H):
                    nc.gpsimd.dma_start(s_bf[0:1, h * 512:h * 512 + 256],
                                        srec_b[h:h + 1, 0:256])

                ln_group(k_d, range(8, 12), "k")
                ln_group(k_d, range(12, 16), "k")
                ln_group(v_d, range(8, 12), "v")
                ln_group(v_d, range(12, 16), "v")

                nc.sync.dma_start(cpwT[:], cpw_d.ap().rearrange("(ct p) o -> p ct o", p=128))

                for h in range(H):
                    attn_pair(h, 16, 256, 512, 12)

                # pass-2 denominator half (short tail)
                nc.vector.reciprocal(s_all[:, 256:512], s_all[:, 256:512])
                nc.vector.tensor_copy(srec_b[:, 256:512], s_all[:, 256:512])
                for h in range(H):
                    nc.gpsimd.dma_start(s_bf[0:1, h * 512 + 256:h * 512 + 512],
                                        srec_b[h:h + 1, 256:512])

        # ---- y scale + c_proj + residual -> xT ----
        with (
            tc.tile_pool(name="pc", bufs=1) as pcp,
            tc.tile_pool(name="pcw", bufs=3) as cw,
            tc.tile_pool(name="pcps", bufs=2, space="PSUM") as cps,
        ):
            ysc = pcp.tile([128, NC, 512], BF16)
            for ct in range(NC):
                rb = cps.tile([128, 512], F32, tag="rb")
                for half in range(2):
                    h = ct * 2 + half
                    for (c0, c1) in ((0, 256), (256, 512)):
                        nc.tensor.matmul(
                            rb[half * 64:half * 64 + 64, c0:c1],
                            w1_bf[0:1, h * 64:h * 64 + 64],
                            s_bf[0:1, h * 512 + c0:h * 512 + c1],
                            tile_position=(0, half * 64),
                            skip_group_check=True)
                t1 = cw.tile([128, 512], F32, tag="yt1")
                nc.vector.tensor_tensor(t1[:], yT_all[:, ct, :], rb[:], ALU.mult)
                nc.vector.tensor_scalar(ysc[:, ct, :], t1[:], 1.0, ln1b(ct),
                                        ALU.mult, ALU.add)
            for ot in range(NC):
                pj = cps.tile([128, 512], F32, tag="cp")
                for ct in range(NC):
                    nc.tensor.matmul(pj[:], cpwT[:, ct, ot * 128:(ot + 1) * 128],
                                     ysc[:, ct, :], start=(ct == 0),
                                     stop=(ct == NC - 1))
                t2 = cw.tile([128, 512], F32, tag="cpt")
                nc.scalar.activation(t2[:], pj[:], AF.Identity, bias=apb(ot))
                nc.vector.tensor_tensor(xT[:, ot, :], t2[:], qT[:, ot, :], ALU.add)

        py_cm.__exit__(None, None, None)

        # ================= Phase 3: LN2 + MLP =================
        with (
            tc.tile_pool(name="pm", bufs=1) as pm,
            tc.tile_pool(name="pmw", bufs=4) as mw,
            tc.tile_pool(name="pms", bufs=1, space="PSUM") as mps,
            tc.tile_pool(name="pma", bufs=2, space="PSUM") as mac,
        ):
            # LN2 stats via PE ones-reductions on a bf16 shadow of xT
            xTb = pm.tile([128, NC, 512], BF16)
            for ct in range(NC):
                nc.vector.tensor_copy(xTb[:, ct, :], xT[:, ct, :])
            s1 = mps.tile([1, 512], F32, tag="s1")
            s2 = mps.tile([1, 512], F32, tag="s2")
            for ct in range(NC):
                nc.tensor.matmul(s1[:], ones_bf[:], xTb[:, ct, :],
                                 start=(ct == 0), stop=(ct == NC - 1),
                                 skip_group_check=True)
            for ct in range(NC):
                sq = mw.tile([128, 512], BF16, tag="sq")
                nc.scalar.activation(sq[:], xTb[:, ct, :], AF.Square)
                nc.tensor.matmul(s2[:], ones_bf[:], sq[:],
                                 start=(ct == 0), stop=(ct == NC - 1),
                                 skip_group_check=True)
            mu = pm.tile([1, 512], F32)
            nc.vector.tensor_scalar(mu[:], s1[:], 1.0 / C, None, ALU.mult)
            ex2 = pm.tile([1, 512], F32)
            nc.vector.tensor_scalar(ex2[:], s2[:], 1.0 / C, EPS, ALU.mult, ALU.add)
            var = pm.tile([1, 512], F32)
            nc.vector.tensor_tensor(var[:], mu[:], mu[:], ALU.mult)
            nc.vector.tensor_tensor(var[:], ex2[:], var[:], ALU.subtract)
            rstd2 = pm.tile([1, 512], F32)
            nc.scalar.activation(rstd2[:], var[:], AF.Sqrt)
            nc.vector.reciprocal(rstd2[:], rstd2[:])
            nmr2 = pm.tile([1, 512], F32)
            nc.vector.tensor_tensor(nmr2[:], mu[:], rstd2[:], ALU.mult)
            nc.vector.tensor_scalar(nmr2[:], nmr2[:], -1.0, None, ALU.mult)

            rstd2b = pm.tile([1, 512], BF16)
            nc.vector.tensor_copy(rstd2b[:], rstd2[:])
            nmr2b = pm.tile([1, 512], BF16)
            nc.vector.tensor_copy(nmr2b[:], nmr2[:])
            ones_bcol = pm.tile([1, 128], BF16)
            nc.gpsimd.memset(ones_bcol[:], 1.0)

            # broadcast rstd2 / -mu*rstd2 to all partitions once via PE
            zA = mps.tile([128, 512], F32, tag="zA")
            zB = mps.tile([128, 512], F32, tag="zB")
            nc.tensor.matmul(zA[:], ones_bcol[:], rstd2b[:], skip_group_check=True)
            nc.tensor.matmul(zB[:], ones_bcol[:], nmr2b[:], skip_group_check=True)

            # z2 = (x * rstd_bc + nmr_bc) * w2[c] + b2[c], bf16
            z2 = pm.tile([128, NC, 512], BF16)
            for ct in range(NC):
                t1 = mw.tile([128, 512], F32, tag="z2t")
                nc.vector.tensor_tensor(t1[:], xT[:, ct, :], zA[:], ALU.mult)
                nc.vector.tensor_tensor(t1[:], t1[:], zB[:], ALU.add)
                nc.vector.tensor_scalar(z2[:, ct, :], t1[:], w2sb[:, ct, 0:1],
                                        w2sb[:, ct, 1:2], ALU.mult, ALU.add)

            # fc + gelu -> mid (bf16)
            mid = pm.tile([128, NF, 512], BF16)
            for ft in range(NF):
                fw = mw.tile([128, NC, 128], BF16, tag="fw")
                nc.sync.dma_start(fw[:], fcw_d.ap()[:, ft * 128:(ft + 1) * 128]
                                  .rearrange("(ct p) f -> p ct f", p=128))
                fp = mac.tile([128, 512], F32, tag="acc")
                for ct in range(NC):
                    nc.tensor.matmul(fp[:], fw[:, ct, :], z2[:, ct, :],
                                     start=(ct == 0), stop=(ct == NC - 1))
                nc.scalar.activation(mid[:, ft, :], fp[:], AF.Gelu_apprx_tanh,
                                     bias=fcb[:, ft:ft + 1])

            # proj + pjb + residual -> outT (reuses qT storage)
            outT = qT
            for ot in range(NC):
                pw = mw.tile([128, NF, 128], BF16, tag="pw")
                nc.sync.dma_start(pw[:], pjw_d.ap()[:, ot * 128:(ot + 1) * 128]
                                  .rearrange("(ft p) f -> p ft f", p=128))
                pacc = mac.tile([128, 512], F32, tag="acc")
                for ft in range(NF):
                    nc.tensor.matmul(pacc[:], pw[:, ft, :], mid[:, ft, :],
                                     start=(ft == 0), stop=(ft == NF - 1))
                t3 = mw.tile([128, 512], F32, tag="ot3")
                nc.vector.tensor_scalar(t3[:], pacc[:], 1.0, pjb(ot),
                                        ALU.mult, ALU.add)
                nc.vector.tensor_tensor(outT[:, ot, :], t3[:], xT[:, ot, :], ALU.add)

            # transpose back to token-major and store
            for i in range(NSLOT):
                on = mw.tile([128, C], F32, tag="onat")
                for og in range(2):
                    po = mac.tile([128, 512], F32, tag="po")
                    for j in range(4):
                        ot = og * 4 + j
                        nc.tensor.transpose(po[:, j * 128:(j + 1) * 128],
                                            outT[:, ot, i * 128:(i + 1) * 128],
                                            ident[:])
                    nc.scalar.copy(on[:, og * 512:(og + 1) * 512], po[:])
                nc.sync.dma_start(out_d.ap()[i], on[:])

    nc.compile()
    return nc


def _host_prep(inputs):
    q = np.asarray(inputs["q"], np.float32)
    k = np.asarray(inputs["k"], np.float32)
    v = np.asarray(inputs["v"], np.float32)
    cpw_t = np.ascontiguousarray(np.asarray(inputs["attn_proj_w"], np.float32).T
                                 ).astype(ml_dtypes.bfloat16)
    fcw_t = np.ascontiguousarray(np.asarray(inputs["fc_w"], np.float32).T
                                 ).astype(ml_dtypes.bfloat16)
    pjw_t = np.ascontiguousarray(np.asarray(inputs["proj_w"], np.float32).T
                                 ).astype(ml_dtypes.bfloat16)
    vecs = np.ascontiguousarray(np.stack(
        [np.asarray(inputs["ln1_w"], np.float32),
         np.asarray(inputs["ln1_b"], np.float32),
         np.asarray(inputs["attn_proj_b"], np.float32),
         np.asarray(inputs["proj_b"], np.float32)], axis=1))
    w2b2f = np.ascontiguousarray(np.stack(
        [np.asarray(inputs["ln2_w"], np.float32),
         np.asarray(inputs["ln2_b"], np.float32)], axis=0))
    fcb = np.ascontiguousarray(np.asarray(inputs["fc_b"], np.float32))

    tri = (np.arange(128)[:, None] <= np.arange(128)[None, :])  # keep tk<=tq

    in_maps, slot_map = [], []
    for c in range(N_CORES):
        b, r = c // 4, c % 4
        slots = [r, 7 - r, 8 + r, 15 - r]
        slot_map.append((b, slots))
        qs = q[b].reshape(NT, 128, C)[slots]
        mask = np.zeros((128, NSLOT, 4, 128), np.float32)
        for i, a in enumerate(slots):
            for p4 in range(4):
                p = 4 * i + p4
                if p < a:
                    mask[:, i, p4, :] = 1.0
                elif p == a:
                    mask[:, i, p4, :] = tri
        in_maps.append({
            "w2b2f": w2b2f,
            "q_s": np.ascontiguousarray(qs).astype(ml_dtypes.bfloat16),
            "k_f": np.ascontiguousarray(k[b].reshape(NT, 128, C)).astype(ml_dtypes.bfloat16),
            "v_f": np.ascontiguousarray(v[b].reshape(NT, 128, C)).astype(ml_dtypes.bfloat16),
            "mask": mask.astype(ml_dtypes.bfloat16),
            "cpw_t": cpw_t, "fcw_t": fcw_t, "pjw_t": pjw_t,
            "vecs": vecs, "fcb": fcb,
        })
    return in_maps, slot_map


def kernel(**inputs):
    if "nc" not in _CACHE:
        _CACHE["nc"] = build()
    nc = _CACHE["nc"]
    in_maps, slot_map = _host_prep(inputs)
    res = run_bass_kernel_spmd(nc, in_maps, core_ids=list(range(N_CORES)))
    out = np.empty((B, T, C), np.float32)
    for c in range(N_CORES):
        b, slots = slot_map[c]
        o = res.results[c]["out"]
        for i, a in enumerate(slots):
            out[b, a * 128:(a + 1) * 128, :] = o[i]
    return out


# revision 25
# speedup vs baseline: 1.3613x; 1.0584x over previous
"""Trainium2 Bass kernel for a dense pre-LN transformer block (B=2, T=2048, C=1024, H=16).

Sharding: zero-collective sequence parallelism over 8 cores. Core c handles
batch b=c//4 and query tiles {r, 7-r, 8+r, 15-r} (r=c%4, 128 rows each) of
that batch: it computes LN1 on the full k/v of its batch, all 16 attention
heads for its 512 query rows, and the attention projection + full MLP for
those rows. The complementary tile assignment balances causal work, and the
program is identical on every core.

Pipeline: per-group (4-tile) LN stats->normalize->transpose chains feed the
attention passes as soon as their key/value groups land, with PE warmup
bursts to trip the HAM clock gate to 8/8 early. ln1_w is folded into the
q-side operand and ln1_b dropped from the k-side (a per-query additive
constant cancels in softmax), so k/v normalization runs on the Vector engine
in bf16 4x mode. Softmax denominators are gathered per head-pair during the
second attention pass. MLP weights stream on the Sync DMA queue while small
gathers use the GpSimd (SWDGE) queue to avoid head-of-line blocking.
"""

import sys

sys.path.insert(0, "/opt/trn_rl_repo")

import numpy as np
import ml_dtypes

import concourse.bass as bass
import concourse.bacc as bacc
import concourse.mybir as mybir
import concourse.tile as tile
from concourse.bass_utils import run_bass_kernel_spmd

F32 = mybir.dt.float32
BF16 = mybir.dt.bfloat16
AF = mybir.ActivationFunctionType
ALU = mybir.AluOpType

B, T, C, H, D = 2, 2048, 1024, 16, 64
NT = T // 128          # 16 key tiles
NC = C // 128          # 8 channel tiles
NF = 4 * C // 128      # 32 fc tiles
NSLOT = 4              # query tiles per core
N_CORES = 8
EPS = 1e-5
SCALE = 1.0 / 8.0      # 1/sqrt(D)

_CACHE = {}


def build():
    nc = bacc.Bacc("TRN2", target_bir_lowering=False, debug=False,
                   num_devices=N_CORES)

    q_d = nc.dram_tensor("q_s", [NSLOT, 128, C], BF16, kind="ExternalInput")
    k_d = nc.dram_tensor("k_f", [NT, 128, C], BF16, kind="ExternalInput")
    v_d = nc.dram_tensor("v_f", [NT, 128, C], BF16, kind="ExternalInput")
    mask_d = nc.dram_tensor("mask", [128, NSLOT, 4, 128], BF16, kind="ExternalInput")
    cpw_d = nc.dram_tensor("cpw_t", [C, C], BF16, kind="ExternalInput")
    fcw_d = nc.dram_tensor("fcw_t", [C, 4 * C], BF16, kind="ExternalInput")
    pjw_d = nc.dram_tensor("pjw_t", [4 * C, C], BF16, kind="ExternalInput")
    vecs_d = nc.dram_tensor("vecs", [C, 4], F32, kind="ExternalInput")
    fcb_d = nc.dram_tensor("fcb", [4 * C], F32, kind="ExternalInput")
    w2f_d = nc.dram_tensor("w2b2f", [2, C], F32, kind="ExternalInput")
    out_d = nc.dram_tensor("out", [NSLOT, 128, C], F32, kind="ExternalOutput")

    with tile.TileContext(nc) as tc:
      with tc.tile_pool(name="pg", bufs=1) as pg:
        # ---- small constants / vectors (live whole kernel) ----
        vecs = pg.tile([128, NC, 4], F32)     # cols: ln1_w, ln1_b, apb, pjb
        nc.gpsimd.dma_start(vecs[:], vecs_d.ap().rearrange("(ct p) v -> p ct v", p=128))
        fcb = pg.tile([128, NF], F32)
        nc.gpsimd.dma_start(fcb[:], fcb_d.ap().rearrange("(ft p) -> p ft", p=128))
        w1_bf = pg.tile([1, C], BF16)
        nc.gpsimd.dma_start(w1_bf[:], vecs_d.ap()[:, 0:1].rearrange("c v -> v c"))
        w2sb = pg.tile([128, NC, 2], F32)
        nc.gpsimd.dma_start(w2sb[:, :, 0:1],
                            w2f_d.ap()[0:1, :].rearrange("k (ct p) -> p ct k", p=128))
        nc.gpsimd.dma_start(w2sb[:, :, 1:2],
                            w2f_d.ap()[1:2, :].rearrange("k (ct p) -> p ct k", p=128))

        ones_sb = pg.tile([128, 128], F32)
        nc.gpsimd.memset(ones_sb[:], 1.0)
        ident = pg.tile([128, 128], F32)
        nc.gpsimd.affine_select(ident[:], ones_sb[:], [[1, 128]], ALU.is_equal,
                                0.0, channel_multiplier=-1)
        ones_bf = pg.tile([128, 1], BF16)
        nc.gpsimd.memset(ones_bf[:], 1.0)
        ones128_bf = pg.tile([128, 128], BF16)
        nc.gpsimd.memset(ones128_bf[:], 1.0)
        ident_bf = pg.tile([128, 128], BF16)
        nc.gpsimd.affine_select(ident_bf[:], ones128_bf[:], [[1, 128]], ALU.is_equal,
                                0.0, channel_multiplier=-1)
        ones512_bf = pg.tile([128, 512], BF16)
        nc.gpsimd.memset(ones512_bf[:], 1.0)

        ln1w = lambda ct: vecs[:, ct, 0:1]
        ln1b = lambda ct: vecs[:, ct, 1:2]
        apb = lambda ct: vecs[:, ct, 2:3]
        pjb = lambda ct: vecs[:, ct, 3:4]

        # ---- cross-phase tensors ----
        qT = pg.tile([128, NC, 512], F32)     # LN1(q)^T with w,b (residual; reused as outT)
        qT2 = pg.tile([128, NC, 512], BF16)   # w * LN1(q)^T  (QK rhs; absorbs k-side w)
        xT = pg.tile([128, NC, 512], F32)     # attn residual output (C-major)

        py_cm = tc.tile_pool(name="py", bufs=1)
        py = py_cm.__enter__()
        yT_all = py.tile([128, NC, 512], F32)  # raw attention out (pre 1/s, w1, b1)
        s_all = py.tile([4, 4 * 512], F32)     # denominators: [h%4, (h//4)*512+q]
        srec2 = py.tile([4, 4 * 512], BF16)
        s_bf = py.tile([1, H * 512], BF16)     # denominator reciprocals, head-major
        ysc = py.tile([128, NC, 512], BF16)    # scaled attention out (c_proj rhs)

        with tc.tile_pool(name="pa", bufs=1) as pa:
            kT = pa.tile([128, NC, T], BF16)       # LN1(k)^T, no w/b (folded to q side)
            v_ext = pa.tile([128, NT, H, 65], BF16)  # LN1(v) (no w,b) + ones col
            masks = pa.tile([128, NSLOT, 4, 128], BF16)
            nc.gpsimd.dma_start(masks[:], mask_d.ap())

            # warmup burst A: real-rate bf16 matmuls to trip HAM to 8/8 early
            with tc.tile_pool(name="wps", bufs=1, space="PSUM") as wps:
                wu = wps.tile([128, 512], F32, tag="wu")
                for _ in range(16):
                    nc.tensor.matmul(wu[:], ones128_bf[:], ones512_bf[:],
                                     skip_group_check=True)

            with (
                tc.tile_pool(name="pln", bufs=7) as pl,
                tc.tile_pool(name="plz", bufs=1) as pz,
                tc.tile_pool(name="pls", bufs=2) as pstat,
                tc.tile_pool(name="paw", bufs=2) as aw,
                tc.tile_pool(name="pap", bufs=2, space="PSUM") as aps,
            ):
                def ln_group(src_d, tts, kind):
                    # load 4 tiles, stats, batched rsqrt, normalize (DVE bf16 4x)
                    xs = []
                    for tt in tts:
                        x_in = pl.tile([128, C], BF16, tag="ln_kv")
                        nc.sync.dma_start(x_in[:], src_d.ap()[tt])
                        xs.append(x_in)
                    aggr = pstat.tile([128, 4, 2], F32, tag="aggr")
                    for gi in range(4):
                        st2 = pstat.tile([128, 2, 6], F32, tag="st2")
                        nc.vector.bn_stats(st2[:, 0, :], xs[gi][:, 0:512])
                        nc.vector.bn_stats(st2[:, 1, :], xs[gi][:, 512:1024])
                        nc.vector.bn_aggr(aggr[:, gi, :], st2[:])
                    veps = pstat.tile([128, 4], F32, tag="veps")
                    lnv = pstat.tile([128, 4], F32, tag="lnv")
                    rstd = pstat.tile([128, 4], F32, tag="rstd")
                    nmr = pstat.tile([128, 4], F32, tag="nmr")
                    nc.vector.tensor_scalar(veps[:], aggr[:, :, 1], EPS, None, ALU.add)
                    # rstd = v^-0.5 via ln/exp: both live in one ACT table set,
                    # so the exp set never swaps out during attention
                    nc.scalar.activation(lnv[:], veps[:], AF.Ln)
                    nc.scalar.activation(rstd[:], lnv[:], AF.Exp, scale=-0.5)
                    nc.vector.tensor_tensor(nmr[:], aggr[:, :, 0], rstd[:], ALU.mult)
                    nc.vector.tensor_scalar(nmr[:], nmr[:], -1.0, None, ALU.mult)

                    if kind == "v":
                        for gi, tt in enumerate(tts):
                            nc.gpsimd.memset(v_ext[:, tt, :, 64:65], 1.0)
                            nc.vector.tensor_scalar(
                                v_ext[:, tt, :, 0:64],
                                xs[gi][:].rearrange("p (h d) -> p h d", h=H),
                                rstd[:, gi:gi + 1], nmr[:, gi:gi + 1],
                                ALU.mult, ALU.add)
                        return

                    zs = []
                    for gi in range(4):
                        z = pz.tile([128, C], BF16, tag=f"z{gi}")
                        nc.vector.tensor_scalar(z[:], xs[gi][:], rstd[:, gi:gi + 1],
                                                nmr[:, gi:gi + 1], ALU.mult, ALU.add)
                        zs.append(z)
                    for ct in range(NC):
                        ps = aps.tile([128, 4, 128], F32, tag="tp")
                        pv = ps[:].bitcast(BF16)[:, :, 0:128]
                        for gi in range(4):
                            nc.tensor.transpose(pv[:, gi, :],
                                                zs[gi][:, ct * 128:(ct + 1) * 128],
                                                ident_bf[:])
                        if kind == "q":
                            nc.scalar.activation(qT[:, ct, :], pv[:], AF.Identity,
                                                 bias=ln1b(ct), scale=ln1w(ct))
                            nc.scalar.activation(qT2[:, ct, :], qT[:, ct, :],
                                                 AF.Identity, scale=ln1w(ct))
                        else:  # k
                            nc.any.tensor_copy(
                                kT[:, ct, tts[0] * 128:(tts[0] + 4) * 128], pv[:])

                def attn_pair(h, np_, c0, c1, nfrom):
                    ct, sel = h // 2, (h % 2) * 64
                    yp = aps.tile([65, 256], F32, tag="yp")
                    for ch in range(np_ // 4):
                        pbase = ch * 4
                        off = 0 if pbase < nfrom else 128
                        sc = aps.tile([128, 4, 256], F32, tag="sc")
                        att = aw.tile([128, 4, 256], BF16, tag="att")
                        for pc in range(4):
                            p = pbase + pc
                            nc.tensor.matmul(
                                sc[:, pc, :],
                                kT[sel:sel + 64, ct, p * 128:(p + 1) * 128],
                                qT2[sel:sel + 64, ct, c0:c1],
                                tile_position=(sel, 0),
                                skip_group_check=True)
                        nc.scalar.activation(att[:, :, off:256], sc[:, :, off:256],
                                             AF.Exp, scale=SCALE)
                        for i in range(NSLOT):
                            if c0 <= i * 128 < c1 and i * 4 == pbase:
                                acol = i * 128 - c0
                                nc.vector.tensor_tensor(
                                    att[:, :, acol:acol + 128],
                                    att[:, :, acol:acol + 128],
                                    masks[:, i, :, :],
                                    ALU.mult)
                        for pc in range(4):
                            p = pbase + pc
                            nc.tensor.matmul(
                                yp[:, off:256],
                                v_ext[:, p, h, :],
                                att[:, pc, off:256],
                                start=(p == 0), stop=(p == np_ - 1),
                                skip_group_check=True)
                    st = aw.tile([65, 256], F32, tag="sst")
                    nc.vector.tensor_copy(st[:, :], yp[:, :])
                    nc.vector.tensor_copy(yT_all[sel:sel + 64, ct, c0:c1],
                                          st[0:64, :])
                    nc.gpsimd.dma_start(
                        s_all[h % 4:h % 4 + 1, (h // 4) * 512 + c0:(h // 4) * 512 + c1],
                        st[64:65, :])

                # ---- pipeline ----
                ln_group(q_d, range(NSLOT), "q")
                ln_group(k_d, range(0, 4), "k")
                ln_group(k_d, range(4, 8), "k")
                ln_group(v_d, range(0, 4), "v")
                ln_group(v_d, range(4, 8), "v")

                # warmup burst B: re-trip HAM right before the QK/AV stream
                wub = aps.tile([128, 4, 256], F32, tag="sc")
                for _ in range(20):
                    nc.tensor.matmul(wub[:, 0:2, :], ones128_bf[:], ones512_bf[:],
                                     skip_group_check=True)

                for h in range(H):
                    attn_pair(h, 8, 0, 256, 4)

                # pass-1 denominator quads: reciprocal+gather overlap pass 2
                for g in range(4):
                    a = g * 512
                    nc.vector.reciprocal(s_all[0:4, a:a + 256], s_all[0:4, a:a + 256])
                    nc.vector.tensor_copy(srec2[0:4, a:a + 256], s_all[0:4, a:a + 256])
                    for r in range(4):
                        h = 4 * g + r
                        nc.gpsimd.dma_start(s_bf[0:1, h * 512:h * 512 + 256],
                                            srec2[r:r + 1, a:a + 256])

                ln_group(k_d, range(8, 12), "k")
                ln_group(k_d, range(12, 16), "k")
                ln_group(v_d, range(8, 12), "v")
                ln_group(v_d, range(12, 16), "v")

                for h in range(H):
                    attn_pair(h, 16, 256, 512, 12)

                # pass-2 denominators + y-scale, pipelined per head quad: only
                # the last quad's chain trails the final attention chunk
                for g in range(4):
                    a = g * 512 + 256
                    nc.vector.reciprocal(s_all[0:4, a:a + 256], s_all[0:4, a:a + 256])
                    nc.vector.tensor_copy(srec2[0:4, a:a + 256], s_all[0:4, a:a + 256])
                    for r in range(4):
                        h = 4 * g + r
                        nc.gpsimd.dma_start(
                            s_bf[0:1, h * 512 + 256:h * 512 + 512],
                            srec2[r:r + 1, a:a + 256])
                    for j in (2 * g, 2 * g + 1):
                        rb = aps.tile([128, 512], F32, tag="tp")
                        for half in range(2):
                            hh = j * 2 + half
                            for (c0, c1) in ((0, 256), (256, 512)):
                                nc.tensor.matmul(
                                    rb[half * 64:half * 64 + 64, c0:c1],
                                    w1_bf[0:1, hh * 64:hh * 64 + 64],
                                    s_bf[0:1, hh * 512 + c0:hh * 512 + c1],
                                    tile_position=(0, half * 64),
                                    skip_group_check=True)
                        t1 = aw.tile([128, 512], F32, tag="t1")
                        nc.vector.tensor_tensor(t1[:], yT_all[:, j, :], rb[:], ALU.mult)
                        nc.vector.tensor_scalar(ysc[:, j, :], t1[:], 1.0, ln1b(j),
                                                ALU.mult, ALU.add)

        # ---- y scale + c_proj + residual -> xT ----
        with (
            tc.tile_pool(name="pcw", bufs=3) as cw,
            tc.tile_pool(name="pcps", bufs=2, space="PSUM") as cps,
        ):
            for ot in range(NC):
                cpw_ot = cw.tile([128, NC, 128], BF16, tag="cpw")
                nc.sync.dma_start(cpw_ot[:], cpw_d.ap()[:, ot * 128:(ot + 1) * 128]
                                  .rearrange("(ct p) o -> p ct o", p=128))
                pj = cps.tile([128, 512], F32, tag="cp")
                for ct in range(NC):
                    nc.tensor.matmul(pj[:], cpw_ot[:, ct, :],
                                     ysc[:, ct, :], start=(ct == 0),
                                     stop=(ct == NC - 1))
                t2 = cw.tile([128, 512], F32, tag="cpt")
                nc.scalar.activation(t2[:], pj[:], AF.Identity, bias=apb(ot))
                nc.vector.tensor_tensor(xT[:, ot, :], t2[:], qT[:, ot, :], ALU.add)

        py_cm.__exit__(None, None, None)

        # ================= Phase 3: LN2 + MLP =================
        with (
            tc.tile_pool(name="pm", bufs=1) as pm,
            tc.tile_pool(name="pmw", bufs=4) as mw,
            tc.tile_pool(name="pms", bufs=1, space="PSUM") as mps,
            tc.tile_pool(name="pma", bufs=2, space="PSUM") as mac,
        ):
            # LN2 stats via PE ones-reductions on a bf16 shadow of xT
            xTb = pm.tile([128, NC, 512], BF16)
            for ct in range(NC):
                nc.vector.tensor_copy(xTb[:, ct, :], xT[:, ct, :])
            s1 = mps.tile([1, 512], F32, tag="s1")
            s2 = mps.tile([1, 512], F32, tag="s2")
            for ct in range(NC):
                nc.tensor.matmul(s1[:], ones_bf[:], xTb[:, ct, :],
                                 start=(ct == 0), stop=(ct == NC - 1),
                                 skip_group_check=True)
            for ct in range(NC):
                sq = mw.tile([128, 512], BF16, tag="sq")
                nc.scalar.activation(sq[:], xTb[:, ct, :], AF.Square)
                nc.tensor.matmul(s2[:], ones_bf[:], sq[:],
                                 start=(ct == 0), stop=(ct == NC - 1),
                                 skip_group_check=True)
            mu = pm.tile([1, 512], F32)
            nc.vector.tensor_scalar(mu[:], s1[:], 1.0 / C, None, ALU.mult)
            ex2 = pm.tile([1, 512], F32)
            nc.vector.tensor_scalar(ex2[:], s2[:], 1.0 / C, EPS, ALU.mult, ALU.add)
            var = pm.tile([1, 512], F32)
            nc.vector.tensor_tensor(var[:], mu[:], mu[:], ALU.mult)
            nc.vector.tensor_tensor(var[:], ex2[:], var[:], ALU.subtract)
            lnv2 = pm.tile([1, 512], F32)
            nc.scalar.activation(lnv2[:], var[:], AF.Ln)
            rstd2 = pm.tile([1, 512], F32)
            nc.scalar.activation(rstd2[:], lnv2[:], AF.Exp, scale=-0.5)
            nmr2 = pm.tile([1, 512], F32)
            nc.vector.tensor_tensor(nmr2[:], mu[:], rstd2[:], ALU.mult)
            nc.vector.tensor_scalar(nmr2[:], nmr2[:], -1.0, None, ALU.mult)

            rstd2b = pm.tile([1, 512], BF16)
            nc.vector.tensor_copy(rstd2b[:], rstd2[:])
            nmr2b = pm.tile([1, 512], BF16)
            nc.vector.tensor_copy(nmr2b[:], nmr2[:])
            ones_bcol = pm.tile([1, 128], BF16)
            nc.gpsimd.memset(ones_bcol[:], 1.0)

            # broadcast rstd2 / -mu*rstd2 to all partitions once via PE
            zA = mps.tile([128, 512], F32, tag="zA")
            zB = mps.tile([128, 512], F32, tag="zB")
            nc.tensor.matmul(zA[:], ones_bcol[:], rstd2b[:], skip_group_check=True)
            nc.tensor.matmul(zB[:], ones_bcol[:], nmr2b[:], skip_group_check=True)

            # z2 = (x * rstd_bc + nmr_bc) * w2[c] + b2[c], bf16
            z2 = pm.tile([128, NC, 512], BF16)
            for ct in range(NC):
                t1 = mw.tile([128, 512], F32, tag="z2t")
                nc.vector.tensor_tensor(t1[:], xT[:, ct, :], zA[:], ALU.mult)
                nc.vector.tensor_tensor(t1[:], t1[:], zB[:], ALU.add)
                nc.vector.tensor_scalar(z2[:, ct, :], t1[:], w2sb[:, ct, 0:1],
                                        w2sb[:, ct, 1:2], ALU.mult, ALU.add)

            # fc + gelu -> mid (bf16)
            mid = pm.tile([128, NF, 512], BF16)
            for ft in range(NF):
                fw = mw.tile([128, NC, 128], BF16, tag="fw")
                nc.sync.dma_start(fw[:], fcw_d.ap()[:, ft * 128:(ft + 1) * 128]
                                  .rearrange("(ct p) f -> p ct f", p=128))
                fp = mac.tile([128, 512], F32, tag="acc")
                for ct in range(NC):
                    nc.tensor.matmul(fp[:], fw[:, ct, :], z2[:, ct, :],
                                     start=(ct == 0), stop=(ct == NC - 1))
                nc.scalar.activation(mid[:, ft, :], fp[:], AF.Gelu_apprx_tanh,
                                     bias=fcb[:, ft:ft + 1])

            # proj + pjb + residual -> outT (reuses qT storage)
            outT = qT
            for ot in range(NC):
                pw = mw.tile([128, NF, 128], BF16, tag="pw")
                nc.sync.dma_start(pw[:], pjw_d.ap()[:, ot * 128:(ot + 1) * 128]
                                  .rearrange("(ft p) f -> p ft f", p=128))
                pacc = mac.tile([128, 512], F32, tag="acc")
                for ft in range(NF):
                    nc.tensor.matmul(pacc[:], pw[:, ft, :], mid[:, ft, :],
                                     start=(ft == 0), stop=(ft == NF - 1))
                t3 = mw.tile([128, 512], F32, tag="ot3")
                nc.vector.tensor_scalar(t3[:], pacc[:], 1.0, pjb(ot),
                                        ALU.mult, ALU.add)
                nc.vector.tensor_tensor(outT[:, ot, :], t3[:], xT[:, ot, :], ALU.add)

            # transpose back to token-major and store
            for i in range(NSLOT):
                on = mw.tile([128, C], F32, tag="onat")
                for og in range(2):
                    po = mac.tile([128, 512], F32, tag="po")
                    for j in range(4):
                        ot = og * 4 + j
                        nc.tensor.transpose(po[:, j * 128:(j + 1) * 128],
                                            outT[:, ot, i * 128:(i + 1) * 128],
                                            ident[:])
                    nc.scalar.copy(on[:, og * 512:(og + 1) * 512], po[:])
                nc.sync.dma_start(out_d.ap()[i], on[:])

    nc.compile()
    return nc


def _host_prep(inputs):
    q = np.asarray(inputs["q"], np.float32)
    k = np.asarray(inputs["k"], np.float32)
    v = np.asarray(inputs["v"], np.float32)
    cpw_t = np.ascontiguousarray(np.asarray(inputs["attn_proj_w"], np.float32).T
                                 ).astype(ml_dtypes.bfloat16)
    fcw_t = np.ascontiguousarray(np.asarray(inputs["fc_w"], np.float32).T
                                 ).astype(ml_dtypes.bfloat16)
    pjw_t = np.ascontiguousarray(np.asarray(inputs["proj_w"], np.float32).T
                                 ).astype(ml_dtypes.bfloat16)
    vecs = np.ascontiguousarray(np.stack(
        [np.asarray(inputs["ln1_w"], np.float32),
         np.asarray(inputs["ln1_b"], np.float32),
         np.asarray(inputs["attn_proj_b"], np.float32),
         np.asarray(inputs["proj_b"], np.float32)], axis=1))
    w2b2f = np.ascontiguousarray(np.stack(
        [np.asarray(inputs["ln2_w"], np.float32),
         np.asarray(inputs["ln2_b"], np.float32)], axis=0))
    fcb = np.ascontiguousarray(np.asarray(inputs["fc_b"], np.float32))

    tri = (np.arange(128)[:, None] <= np.arange(128)[None, :])  # keep tk<=tq

    in_maps, slot_map = [], []
    for c in range(N_CORES):
        b, r = c // 4, c % 4
        slots = [r, 7 - r, 8 + r, 15 - r]
        slot_map.append((b, slots))
        qs = q[b].reshape(NT, 128, C)[slots]
        mask = np.zeros((128, NSLOT, 4, 128), np.float32)
        for i, a in enumerate(slots):
            for p4 in range(4):
                p = 4 * i + p4
                if p < a:
                    mask[:, i, p4, :] = 1.0
                elif p == a:
                    mask[:, i, p4, :] = tri
        in_maps.append({
            "w2b2f": w2b2f,
            "q_s": np.ascontiguousarray(qs).astype(ml_dtypes.bfloat16),
            "k_f": np.ascontiguousarray(k[b].reshape(NT, 128, C)).astype(ml_dtypes.bfloat16),
            "v_f": np.ascontiguousarray(v[b].reshape(NT, 128, C)).astype(ml_dtypes.bfloat16),
            "mask": mask.astype(ml_dtypes.bfloat16),
            "cpw_t": cpw_t, "fcw_t": fcw_t, "pjw_t": pjw_t,
            "vecs": vecs, "fcb": fcb,
        })
    return in_maps, slot_map


def kernel(**inputs):
    if "nc" not in _CACHE:
        _CACHE["nc"] = build()
    nc = _CACHE["nc"]
    in_maps, slot_map = _host_prep(inputs)
    res = run_bass_kernel_spmd(nc, in_maps, core_ids=list(range(N_CORES)))
    out = np.empty((B, T, C), np.float32)
    for c in range(N_CORES):
        b, slots = slot_map[c]
        o = res.results[c]["out"]
        for i, a in enumerate(slots):
            out[b, a * 128:(a + 1) * 128, :] = o[i]
    return out


# revision 33
# speedup vs baseline: 1.4193x; 1.0426x over previous
"""Trainium2 Bass kernel for a dense pre-LN transformer block (B=2, T=2048, C=1024, H=16).

Sharding: zero-collective sequence parallelism over 8 cores. Core c handles
batch b=c//4 and query tiles {r, 7-r, 8+r, 15-r} (r=c%4, 128 rows each) of
that batch: it computes LN1 on the full k/v of its batch, all 16 attention
heads for its 512 query rows, and the attention projection + full MLP for
those rows. The complementary tile assignment balances causal work, and the
program is identical on every core.

Pipeline: per-group (4-tile) LN stats->normalize->transpose chains feed the
attention passes as soon as their key/value groups land, with PE warmup
bursts to trip the HAM clock gate to 8/8 early. ln1_w is folded into the
q-side operand and ln1_b dropped from the k-side (a per-query additive
constant cancels in softmax), so k/v normalization runs on the Vector engine
in bf16 4x mode. Softmax denominators are gathered per head-pair during the
second attention pass. MLP weights stream on the Sync DMA queue while small
gathers use the GpSimd (SWDGE) queue to avoid head-of-line blocking.
"""

import sys

sys.path.insert(0, "/opt/trn_rl_repo")

import numpy as np
import ml_dtypes

import concourse.bass as bass
import concourse.bacc as bacc
import concourse.mybir as mybir
import concourse.tile as tile
from concourse.bass_utils import run_bass_kernel_spmd

F32 = mybir.dt.float32
BF16 = mybir.dt.bfloat16
AF = mybir.ActivationFunctionType
ALU = mybir.AluOpType

B, T, C, H, D = 2, 2048, 1024, 16, 64
NT = T // 128          # 16 key tiles
NC = C // 128          # 8 channel tiles
NF = 4 * C // 128      # 32 fc tiles
NSLOT = 4              # query tiles per core
N_CORES = 8
EPS = 1e-5
SCALE = 1.0 / 8.0      # 1/sqrt(D)

_CACHE = {}


def build():
    nc = bacc.Bacc("TRN2", target_bir_lowering=False, debug=False,
                   num_devices=N_CORES)

    q_d = nc.dram_tensor("q_s", [NSLOT, 128, C], BF16, kind="ExternalInput")
    k_d = nc.dram_tensor("k_f", [NT, 128, C], BF16, kind="ExternalInput")
    v_d = nc.dram_tensor("v_f", [NT, 128, C], BF16, kind="ExternalInput")
    mask_d = nc.dram_tensor("mask", [128, NSLOT, 4, 128], BF16, kind="ExternalInput")
    cpw_d = nc.dram_tensor("cpw_t", [C, C], BF16, kind="ExternalInput")
    fcw_d = nc.dram_tensor("fcw_t", [C, 4 * C], BF16, kind="ExternalInput")
    pjw_d = nc.dram_tensor("pjw_t", [4 * C, C], BF16, kind="ExternalInput")
    vecs_d = nc.dram_tensor("vecs", [C, 4], F32, kind="ExternalInput")
    fcb_d = nc.dram_tensor("fcb", [4 * C], F32, kind="ExternalInput")
    w2f_d = nc.dram_tensor("w2b2f", [2, C], F32, kind="ExternalInput")
    out_d = nc.dram_tensor("out", [NSLOT, 128, C], F32, kind="ExternalOutput")

    with tile.TileContext(nc) as tc:
      with tc.tile_pool(name="pg", bufs=1) as pg:
        # ---- small constants / vectors (live whole kernel) ----
        vecs = pg.tile([128, NC, 4], F32)     # cols: ln1_w, ln1_b, apb, pjb
        nc.gpsimd.dma_start(vecs[:], vecs_d.ap().rearrange("(ct p) v -> p ct v", p=128))
        fcb = pg.tile([128, NF], F32)
        nc.gpsimd.dma_start(fcb[:], fcb_d.ap().rearrange("(ft p) -> p ft", p=128))
        w1_bf = pg.tile([1, C], BF16)
        nc.gpsimd.dma_start(w1_bf[:], vecs_d.ap()[:, 0:1].rearrange("c v -> v c"))
        w2sb = pg.tile([128, NC, 2], F32)
        nc.gpsimd.dma_start(w2sb[:, :, 0:1],
                            w2f_d.ap()[0:1, :].rearrange("k (ct p) -> p ct k", p=128))
        nc.gpsimd.dma_start(w2sb[:, :, 1:2],
                            w2f_d.ap()[1:2, :].rearrange("k (ct p) -> p ct k", p=128))

        ones_sb = pg.tile([128, 128], F32)
        nc.gpsimd.memset(ones_sb[:], 1.0)
        ident = pg.tile([128, 128], F32)
        nc.gpsimd.affine_select(ident[:], ones_sb[:], [[1, 128]], ALU.is_equal,
                                0.0, channel_multiplier=-1)
        ones_bf = pg.tile([128, 1], BF16)
        nc.gpsimd.memset(ones_bf[:], 1.0)
        ones128_bf = pg.tile([128, 128], BF16)
        nc.gpsimd.memset(ones128_bf[:], 1.0)
        ident_bf = pg.tile([128, 128], BF16)
        nc.gpsimd.affine_select(ident_bf[:], ones128_bf[:], [[1, 128]], ALU.is_equal,
                                0.0, channel_multiplier=-1)
        ones512_bf = pg.tile([128, 512], BF16)
        nc.gpsimd.memset(ones512_bf[:], 1.0)

        ln1w = lambda ct: vecs[:, ct, 0:1]
        ln1b = lambda ct: vecs[:, ct, 1:2]
        apb = lambda ct: vecs[:, ct, 2:3]
        pjb = lambda ct: vecs[:, ct, 3:4]

        # ---- cross-phase tensors ----
        qT = pg.tile([128, NC, 512], F32)     # LN1(q)^T with w,b (residual; reused as outT)
        # dual-lane QK rhs: cols 0:512 = w*LN1(q) rows 0:64 (even head, rows
        # 64:128 zero); cols 512:1024 = rows 64:128 (odd head, rows 0:64 zero)
        qT2z = pg.tile([128, NC, 1024], BF16)
        nc.gpsimd.memset(qT2z[:], 0.0)
        xT = pg.tile([128, NC, 512], F32)     # attn residual output (C-major)

        py_cm = tc.tile_pool(name="py", bufs=1)
        py = py_cm.__enter__()
        yT_all = py.tile([128, NC, 512], F32)  # raw attention out (pre 1/s, w1, b1)
        s_all = py.tile([4, 4 * 512], F32)     # denominators: [h%4, (h//4)*512+q]
        srec2 = py.tile([4, 4 * 512], BF16)
        s_bf = py.tile([1, H * 512], BF16)     # denominator reciprocals, head-major
        ysc = py.tile([128, NC, 512], BF16)    # scaled attention out (c_proj rhs)

        with tc.tile_pool(name="pa", bufs=1) as pa:
            kT = pa.tile([128, NC, T], BF16)       # LN1(k)^T, no w/b (folded to q side)
            v_ext = pa.tile([128, NT, H, 65], BF16)  # LN1(v) (no w,b) + ones col
            masks = pa.tile([128, NSLOT, 4, 128], BF16)
            nc.gpsimd.dma_start(masks[:], mask_d.ap())

            # warmup burst A: real-rate bf16 matmuls to trip HAM to 8/8 early
            with tc.tile_pool(name="wps", bufs=1, space="PSUM") as wps:
                wu = wps.tile([128, 512], F32, tag="wu")
                for _ in range(16):
                    nc.tensor.matmul(wu[:], ones128_bf[:], ones512_bf[:],
                                     skip_group_check=True)

            with (
                tc.tile_pool(name="pln", bufs=7) as pl,
                tc.tile_pool(name="plz", bufs=1) as pz,
                tc.tile_pool(name="pls", bufs=2) as pstat,
                tc.tile_pool(name="paw", bufs=2) as aw,
                tc.tile_pool(name="pap", bufs=2, space="PSUM") as aps,
            ):
                def ln_group(src_d, tts, kind):
                    # load 4 tiles, stats, batched rsqrt, normalize (DVE bf16 4x)
                    xs = []
                    for tt in tts:
                        x_in = pl.tile([128, C], BF16, tag="ln_kv")
                        nc.sync.dma_start(x_in[:], src_d.ap()[tt])
                        xs.append(x_in)
                    aggr = pstat.tile([128, 4, 2], F32, tag="aggr")
                    for gi in range(4):
                        st2 = pstat.tile([128, 2, 6], F32, tag="st2")
                        nc.vector.bn_stats(st2[:, 0, :], xs[gi][:, 0:512])
                        nc.vector.bn_stats(st2[:, 1, :], xs[gi][:, 512:1024])
                        nc.vector.bn_aggr(aggr[:, gi, :], st2[:])
                    veps = pstat.tile([128, 4], F32, tag="veps")
                    rstd = pstat.tile([128, 4], F32, tag="rstd")
                    nmr = pstat.tile([128, 4], F32, tag="nmr")
                    nc.vector.tensor_scalar(veps[:], aggr[:, :, 1], EPS, None, ALU.add)
                    nc.scalar.activation(rstd[:], veps[:], AF.Sqrt)
                    nc.vector.reciprocal(rstd[:], rstd[:])
                    nc.vector.tensor_tensor(nmr[:], aggr[:, :, 0], rstd[:], ALU.mult)
                    nc.vector.tensor_scalar(nmr[:], nmr[:], -1.0, None, ALU.mult)

                    if kind == "v":
                        for gi, tt in enumerate(tts):
                            nc.gpsimd.memset(v_ext[:, tt, :, 64:65], 1.0)
                            nc.vector.tensor_scalar(
                                v_ext[:, tt, :, 0:64],
                                xs[gi][:].rearrange("p (h d) -> p h d", h=H),
                                rstd[:, gi:gi + 1], nmr[:, gi:gi + 1],
                                ALU.mult, ALU.add)
                        return

                    zs = []
                    for gi in range(4):
                        z = pz.tile([128, C], BF16, tag=f"z{gi}")
                        nc.vector.tensor_scalar(z[:], xs[gi][:], rstd[:, gi:gi + 1],
                                                nmr[:, gi:gi + 1], ALU.mult, ALU.add)
                        zs.append(z)
                    for ct in range(NC):
                        ps = aps.tile([128, 4, 128], F32, tag="tp")
                        pv = ps[:].bitcast(BF16)[:, :, 0:128]
                        for gi in range(4):
                            nc.tensor.transpose(pv[:, gi, :],
                                                zs[gi][:, ct * 128:(ct + 1) * 128],
                                                ident_bf[:])
                        if kind == "q":
                            nc.scalar.activation(qT[:, ct, :], pv[:], AF.Identity,
                                                 bias=ln1b(ct), scale=ln1w(ct))
                            nc.scalar.activation(qT2z[0:64, ct, 0:512],
                                                 qT[0:64, ct, :], AF.Identity,
                                                 scale=vecs[0:64, ct, 0:1])
                            nc.scalar.activation(qT2z[64:128, ct, 512:1024],
                                                 qT[64:128, ct, :], AF.Identity,
                                                 scale=vecs[64:128, ct, 0:1])
                        else:  # k
                            nc.any.tensor_copy(
                                kT[:, ct, tts[0] * 128:(tts[0] + 4) * 128], pv[:])

                def attn_pair2(ct, np_, c0, c1, nfrom):
                    # heads 2ct (lane 0, rows 0:64) and 2ct+1 (lane 1, rows
                    # 64:128) computed together: full 128-row QK matmuls via
                    # the zero-padded dual-lane q operand qT2z
                    yp0 = aps.tile([65, 256], F32, tag="yp")
                    yp1 = aps.tile([65, 256], F32, tag="yp")
                    q2 = qT2z[:, ct, :].rearrange("p (l q) -> p l q", l=2)[:, :, c0:c1]
                    for ch in range(np_ // 2):
                        pbase = ch * 2
                        off = 0 if pbase < nfrom else 128
                        sc = aps.tile([128, 2, 512], F32, tag="sc")
                        att = aw.tile([128, 2, 512], BF16, tag="att")
                        for pc in range(2):
                            p = pbase + pc
                            nc.tensor.matmul(
                                sc[:, pc, :],
                                kT[:, ct, p * 128:(p + 1) * 128],
                                q2,
                                skip_group_check=True)
                        scv = sc[:].rearrange("k c (l q) -> k c l q", l=2)
                        atv = att[:].rearrange("k c (l q) -> k c l q", l=2)
                        nc.scalar.activation(atv[:, :, :, off:256],
                                             scv[:, :, :, off:256],
                                             AF.Exp, scale=SCALE)
                        for i in range(NSLOT):
                            if c0 <= i * 128 < c1 and pbase in (i * 4, i * 4 + 2):
                                acol = i * 128 - c0
                                pb2 = pbase - i * 4
                                for l in range(2):
                                    nc.vector.tensor_tensor(
                                        atv[:, :, l, acol:acol + 128],
                                        atv[:, :, l, acol:acol + 128],
                                        masks[:, i, pb2:pb2 + 2, :],
                                        ALU.mult)
                        for pc in range(2):
                            p = pbase + pc
                            nc.tensor.matmul(
                                yp0[:, off:256], v_ext[:, p, 2 * ct, :],
                                att[:, pc, off:256],
                                start=(p == 0), stop=(p == np_ - 1),
                                skip_group_check=True)
                            nc.tensor.matmul(
                                yp1[:, off:256], v_ext[:, p, 2 * ct + 1, :],
                                att[:, pc, 256 + off:512],
                                start=(p == 0), stop=(p == np_ - 1),
                                skip_group_check=True)
                    for parity, ypx in ((0, yp0), (1, yp1)):
                        h = 2 * ct + parity
                        sel = parity * 64
                        st = aw.tile([65, 256], F32, tag="sst")
                        nc.vector.tensor_copy(st[:, :], ypx[:, :])
                        nc.vector.tensor_copy(yT_all[sel:sel + 64, ct, c0:c1],
                                              st[0:64, :])
                        nc.gpsimd.dma_start(
                            s_all[h % 4:h % 4 + 1,
                                  (h // 4) * 512 + c0:(h // 4) * 512 + c1],
                            st[64:65, :])

                # ---- pipeline: all LN groups up front (keeps the sqrt ACT
                # table resident in one window; exp set loads once after) ----
                ln_group(q_d, range(NSLOT), "q")
                ln_group(k_d, range(0, 4), "k")
                ln_group(k_d, range(4, 8), "k")
                ln_group(v_d, range(0, 4), "v")
                ln_group(v_d, range(4, 8), "v")
                ln_group(k_d, range(8, 12), "k")
                ln_group(k_d, range(12, 16), "k")
                ln_group(v_d, range(8, 12), "v")
                ln_group(v_d, range(12, 16), "v")

                # warmup burst B: re-trip HAM right before the QK/AV stream
                wub = aps.tile([128, 4, 256], F32, tag="sc")
                for _ in range(20):
                    nc.tensor.matmul(wub[:, 0:2, :], ones128_bf[:], ones512_bf[:],
                                     skip_group_check=True)

                for ctp in range(NC):
                    attn_pair2(ctp, 8, 0, 256, 4)

                # pass-1 denominator quads: reciprocal+gather overlap pass 2
                for g in range(4):
                    a = g * 512
                    nc.vector.reciprocal(s_all[0:4, a:a + 256], s_all[0:4, a:a + 256])
                    nc.vector.tensor_copy(srec2[0:4, a:a + 256], s_all[0:4, a:a + 256])
                    for r in range(4):
                        h = 4 * g + r
                        nc.gpsimd.dma_start(s_bf[0:1, h * 512:h * 512 + 256],
                                            srec2[r:r + 1, a:a + 256])

                for ctp in range(NC):
                    attn_pair2(ctp, 16, 256, 512, 12)

                # pass-2 denominators + y-scale, pipelined per head quad: only
                # the last quad's chain trails the final attention chunk
                for g in range(4):
                    a = g * 512 + 256
                    nc.vector.reciprocal(s_all[0:4, a:a + 256], s_all[0:4, a:a + 256])
                    nc.vector.tensor_copy(srec2[0:4, a:a + 256], s_all[0:4, a:a + 256])
                    for r in range(4):
                        h = 4 * g + r
                        nc.gpsimd.dma_start(
                            s_bf[0:1, h * 512 + 256:h * 512 + 512],
                            srec2[r:r + 1, a:a + 256])
                    for j in (2 * g, 2 * g + 1):
                        rb = aps.tile([128, 512], F32, tag="tp")
                        for half in range(2):
                            hh = j * 2 + half
                            for (c0, c1) in ((0, 256), (256, 512)):
                                nc.tensor.matmul(
                                    rb[half * 64:half * 64 + 64, c0:c1],
                                    w1_bf[0:1, hh * 64:hh * 64 + 64],
                                    s_bf[0:1, hh * 512 + c0:hh * 512 + c1],
                                    tile_position=(0, half * 64),
                                    skip_group_check=True)
                        t1 = aw.tile([128, 512], F32, tag="t1")
                        nc.vector.tensor_tensor(t1[:], yT_all[:, j, :], rb[:], ALU.mult)
                        nc.vector.tensor_scalar(ysc[:, j, :], t1[:], 1.0, ln1b(j),
                                                ALU.mult, ALU.add)

        # ---- y scale + c_proj + residual -> xT ----
        with (
            tc.tile_pool(name="pcw", bufs=3) as cw,
            tc.tile_pool(name="pcps", bufs=2, space="PSUM") as cps,
        ):
            for ot in range(NC):
                cpw_ot = cw.tile([128, NC, 128], BF16, tag="cpw")
                nc.sync.dma_start(cpw_ot[:], cpw_d.ap()[:, ot * 128:(ot + 1) * 128]
                                  .rearrange("(ct p) o -> p ct o", p=128))
                pj = cps.tile([128, 512], F32, tag="cp")
                for ct in range(NC):
                    nc.tensor.matmul(pj[:], cpw_ot[:, ct, :],
                                     ysc[:, ct, :], start=(ct == 0),
                                     stop=(ct == NC - 1))
                t2 = cw.tile([128, 512], F32, tag="cpt")
                nc.scalar.activation(t2[:], pj[:], AF.Identity, bias=apb(ot))
                nc.vector.tensor_tensor(xT[:, ot, :], t2[:], qT[:, ot, :], ALU.add)

        py_cm.__exit__(None, None, None)

        # ================= Phase 3: LN2 + MLP =================
        with (
            tc.tile_pool(name="pm", bufs=1) as pm,
            tc.tile_pool(name="pmw", bufs=4) as mw,
            tc.tile_pool(name="pms", bufs=1, space="PSUM") as mps,
            tc.tile_pool(name="pma", bufs=2, space="PSUM") as mac,
        ):
            # LN2 stats via PE ones-reductions on a bf16 shadow of xT
            xTb = pm.tile([128, NC, 512], BF16)
            for ct in range(NC):
                nc.vector.tensor_copy(xTb[:, ct, :], xT[:, ct, :])
            s1 = mps.tile([1, 512], F32, tag="s1")
            s2 = mps.tile([1, 512], F32, tag="s2")
            for ct in range(NC):
                nc.tensor.matmul(s1[:], ones_bf[:], xTb[:, ct, :],
                                 start=(ct == 0), stop=(ct == NC - 1),
                                 skip_group_check=True)
            for ct in range(NC):
                sq = mw.tile([128, 512], BF16, tag="sq")
                nc.scalar.activation(sq[:], xTb[:, ct, :], AF.Square)
                nc.tensor.matmul(s2[:], ones_bf[:], sq[:],
                                 start=(ct == 0), stop=(ct == NC - 1),
                                 skip_group_check=True)
            mu = pm.tile([1, 512], F32)
            nc.vector.tensor_scalar(mu[:], s1[:], 1.0 / C, None, ALU.mult)
            ex2 = pm.tile([1, 512], F32)
            nc.vector.tensor_scalar(ex2[:], s2[:], 1.0 / C, EPS, ALU.mult, ALU.add)
            var = pm.tile([1, 512], F32)
            nc.vector.tensor_tensor(var[:], mu[:], mu[:], ALU.mult)
            nc.vector.tensor_tensor(var[:], ex2[:], var[:], ALU.subtract)
            rstd2 = pm.tile([1, 512], F32)
            nc.scalar.activation(rstd2[:], var[:], AF.Sqrt)
            nc.vector.reciprocal(rstd2[:], rstd2[:])
            nmr2 = pm.tile([1, 512], F32)
            nc.vector.tensor_tensor(nmr2[:], mu[:], rstd2[:], ALU.mult)
            nc.vector.tensor_scalar(nmr2[:], nmr2[:], -1.0, None, ALU.mult)

            rstd2b = pm.tile([1, 512], BF16)
            nc.vector.tensor_copy(rstd2b[:], rstd2[:])
            nmr2b = pm.tile([1, 512], BF16)
            nc.vector.tensor_copy(nmr2b[:], nmr2[:])
            ones_bcol = pm.tile([1, 128], BF16)
            nc.gpsimd.memset(ones_bcol[:], 1.0)

            # broadcast rstd2 / -mu*rstd2 to all partitions once via PE
            zA = mps.tile([128, 512], F32, tag="zA")
            zB = mps.tile([128, 512], F32, tag="zB")
            nc.tensor.matmul(zA[:], ones_bcol[:], rstd2b[:], skip_group_check=True)
            nc.tensor.matmul(zB[:], ones_bcol[:], nmr2b[:], skip_group_check=True)

            # z2 = (x * rstd_bc + nmr_bc) * w2[c] + b2[c], bf16
            z2 = pm.tile([128, NC, 512], BF16)
            for ct in range(NC):
                t1 = mw.tile([128, 512], F32, tag="z2t")
                nc.vector.tensor_tensor(t1[:], xT[:, ct, :], zA[:], ALU.mult)
                nc.vector.tensor_tensor(t1[:], t1[:], zB[:], ALU.add)
                nc.vector.tensor_scalar(z2[:, ct, :], t1[:], w2sb[:, ct, 0:1],
                                        w2sb[:, ct, 1:2], ALU.mult, ALU.add)

            # fc + gelu -> mid (bf16)
            mid = pm.tile([128, NF, 512], BF16)
            for ft in range(NF):
                fw = mw.tile([128, NC, 128], BF16, tag="fw")
                nc.sync.dma_start(fw[:], fcw_d.ap()[:, ft * 128:(ft + 1) * 128]
                                  .rearrange("(ct p) f -> p ct f", p=128))
                fp = mac.tile([128, 512], F32, tag="acc")
                for ct in range(NC):
                    nc.tensor.matmul(fp[:], fw[:, ct, :], z2[:, ct, :],
                                     start=(ct == 0), stop=(ct == NC - 1))
                nc.scalar.activation(mid[:, ft, :], fp[:], AF.Gelu_apprx_tanh,
                                     bias=fcb[:, ft:ft + 1])

            # proj + pjb + residual -> outT (reuses qT storage)
            outT = qT
            for ot in range(NC):
                pw = mw.tile([128, NF, 128], BF16, tag="pw")
                nc.sync.dma_start(pw[:], pjw_d.ap()[:, ot * 128:(ot + 1) * 128]
                                  .rearrange("(ft p) f -> p ft f", p=128))
                pacc = mac.tile([128, 512], F32, tag="acc")
                for ft in range(NF):
                    nc.tensor.matmul(pacc[:], pw[:, ft, :], mid[:, ft, :],
                                     start=(ft == 0), stop=(ft == NF - 1))
                t3 = mw.tile([128, 512], F32, tag="ot3")
                nc.vector.tensor_scalar(t3[:], pacc[:], 1.0, pjb(ot),
                                        ALU.mult, ALU.add)
                nc.vector.tensor_tensor(outT[:, ot, :], t3[:], xT[:, ot, :], ALU.add)

            # transpose back to token-major and store
            for i in range(NSLOT):
                on = mw.tile([128, C], F32, tag="onat")
                for og in range(2):
                    po = mac.tile([128, 512], F32, tag="po")
                    for j in range(4):
                        ot = og * 4 + j
                        nc.tensor.transpose(po[:, j * 128:(j + 1) * 128],
                                            outT[:, ot, i * 128:(i + 1) * 128],
                                            ident[:])
                    nc.scalar.copy(on[:, og * 512:(og + 1) * 512], po[:])
                nc.sync.dma_start(out_d.ap()[i], on[:])

    nc.compile()
    return nc


def _host_prep(inputs):
    q = np.asarray(inputs["q"], np.float32)
    k = np.asarray(inputs["k"], np.float32)
    v = np.asarray(inputs["v"], np.float32)
    cpw_t = np.ascontiguousarray(np.asarray(inputs["attn_proj_w"], np.float32).T
                                 ).astype(ml_dtypes.bfloat16)
    fcw_t = np.ascontiguousarray(np.asarray(inputs["fc_w"], np.float32).T
                                 ).astype(ml_dtypes.bfloat16)
    pjw_t = np.ascontiguousarray(np.asarray(inputs["proj_w"], np.float32).T
                                 ).astype(ml_dtypes.bfloat16)
    vecs = np.ascontiguousarray(np.stack(
        [np.asarray(inputs["ln1_w"], np.float32),
         np.asarray(inputs["ln1_b"], np.float32),
         np.asarray(inputs["attn_proj_b"], np.float32),
         np.asarray(inputs["proj_b"], np.float32)], axis=1))
    w2b2f = np.ascontiguousarray(np.stack(
        [np.asarray(inputs["ln2_w"], np.float32),
         np.asarray(inputs["ln2_b"], np.float32)], axis=0))
    fcb = np.ascontiguousarray(np.asarray(inputs["fc_b"], np.float32))

    tri = (np.arange(128)[:, None] <= np.arange(128)[None, :])  # keep tk<=tq

    in_maps, slot_map = [], []
    for c in range(N_CORES):
        b, r = c // 4, c % 4
        slots = [r, 7 - r, 8 + r, 15 - r]
        slot_map.append((b, slots))
        qs = q[b].reshape(NT, 128, C)[slots]
        mask = np.zeros((128, NSLOT, 4, 128), np.float32)
        for i, a in enumerate(slots):
            for p4 in range(4):
                p = 4 * i + p4
                if p < a:
                    mask[:, i, p4, :] = 1.0
                elif p == a:
                    mask[:, i, p4, :] = tri
        in_maps.append({
            "w2b2f": w2b2f,
            "q_s": np.ascontiguousarray(qs).astype(ml_dtypes.bfloat16),
            "k_f": np.ascontiguousarray(k[b].reshape(NT, 128, C)).astype(ml_dtypes.bfloat16),
            "v_f": np.ascontiguousarray(v[b].reshape(NT, 128, C)).astype(ml_dtypes.bfloat16),
            "mask": mask.astype(ml_dtypes.bfloat16),
            "cpw_t": cpw_t, "fcw_t": fcw_t, "pjw_t": pjw_t,
            "vecs": vecs, "fcb": fcb,
        })
    return in_maps, slot_map


def kernel(**inputs):
    if "nc" not in _CACHE:
        _CACHE["nc"] = build()
    nc = _CACHE["nc"]
    in_maps, slot_map = _host_prep(inputs)
    res = run_bass_kernel_spmd(nc, in_maps, core_ids=list(range(N_CORES)))
    out = np.empty((B, T, C), np.float32)
    for c in range(N_CORES):
        b, slots = slot_map[c]
        o = res.results[c]["out"]
        for i, a in enumerate(slots):
            out[b, a * 128:(a + 1) * 128, :] = o[i]
    return out
